# revision 22
# baseline (speedup 1.0000x reference)
"""Trainium2 Bass kernel for DiffSelfAttention (B=1, T=2048, C=2048, 16 v-heads).

Sharding: tensor-parallel over heads across 8 NeuronCores. Core c owns
v-heads {2c, 2c+1} plus the matching q/k heads of both differential branches.
Each core computes its qkv slice, the attention for its 4 q/k head-pairs, the
differential + per-head RMSNorm, and a partial projection
y_c = out_c @ w_proj[rows_c]. The host sums the 8 partials (unshard step).

v2 design notes (vs the fp32r v1):
  - Everything bf16 on the PE (1 cycle/row at ANY moving size, halved DMA
    and SBUF footprint). Host converts inputs; rel-err budget is 2e-2 and
    bf16 keeps us ~1e-2 or better.
  - Transposed PV: attention is computed as a^T[tq,d2] = et^T @ [v|1] with
    the exp'd scores as the STATIONARY operand and [v | ones-column] as a
    129-wide moving operand. This gets the softmax denominator r in the
    same matmul (column 128) AND puts r on the partition axis, so all the
    differential-combine scalars are per-partition [P,1] operands — no
    broadcast matmuls. v1's separate ones-colsum (131k cycles/core) is gone.
  - Softmax divisions eliminated as in v1: RMSNorm is invariant to any
    per-column positive scale, so o' = a1*r2 - lam*a2*r1 feeds the norm.
  - RMS rsqrt = exp(-0.5*ln(mean)) on ACT, batched [P,8] per (block,head).
    rms_scale * (1-lambda_init) is folded into w_proj rows on the host.
  - o'[tq,d2] is transposed back to [d2,tq] for the projection with the
    DMA xbar transpose engine (idle mid-kernel), not the PE.
  - Work is emitted in 2 tq-blocks of 1024; the q2 projections for the
    second half and the first block's output projection are injected as
    fillers into the (ACT-bound) attention sweeps so the PE never idles.
  - PSUM budget (8 banks): scores [P,1024]x2 = 4, pv accumulators
    (8 x [P,132] packed 3-per-bank) = 3, proj/filler = 1.
"""

import math

import numpy as np

import concourse.bass as bass
import concourse.bacc as bacc
import concourse.mybir as mybir
import concourse.tile as tile

F32 = mybir.dt.float32
BF16 = mybir.dt.bfloat16

T = 2048
C = 2048
N_HEAD = 16
H_DIM = 64
D2 = 2 * H_DIM  # 128 (v-head dim, also the RMS group size)
LAMBDA_INIT = 0.8 - 0.6 * math.exp(-0.3)
SCALE = 1.0 / math.sqrt(H_DIM)
P = 128
KS = C // P  # 16 contraction slabs
TT = T // P  # 16 t-tiles
NCH = 512  # phase-1 t-chunk width
QS = 8  # tq slabs per block
NBLK = 2  # tq blocks of 1024
N_CORES = 8

EXP = mybir.ActivationFunctionType.Exp
LOG = mybir.ActivationFunctionType.Ln
CPY = mybir.ActivationFunctionType.Copy
MULT = mybir.AluOpType.mult
ADD = mybir.AluOpType.add


def build(lam: float) -> bass.Bass:
    nc = bacc.Bacc("TRN2", target_bir_lowering=False, debug=False)

    xb_d = nc.dram_tensor("xt", [P, 4, KS, NCH], BF16, kind="ExternalInput")
    wqk_d = nc.dram_tensor("wqk", [P, 4, KS, P], BF16, kind="ExternalInput")
    wv_d = nc.dram_tensor("wv", [P, KS, 2 * D2], BF16, kind="ExternalInput")
    wp_d = nc.dram_tensor("wp", [P, 2, T], BF16, kind="ExternalInput")
    id_d = nc.dram_tensor("ident", [P, P], BF16, kind="ExternalInput")
    y_d = nc.dram_tensor("y", [TT, P, T], BF16, kind="ExternalOutput")

    with tile.TileContext(nc) as tc:
        with tc.tile_pool(name="persist", bufs=1) as pp, \
             tc.tile_pool(name="etp", bufs=3) as etp, \
             tc.tile_pool(name="work", bufs=2) as wkp, \
             tc.tile_pool(name="ysp", bufs=2) as ysp, \
             tc.tile_pool(name="sc", bufs=2, space="PSUM") as scp, \
             tc.tile_pool(name="acc", bufs=1, space="PSUM") as accp, \
             tc.tile_pool(name="yp", bufs=1, space="PSUM") as ypp:

            xb = pp.tile([P, 4, KS, NCH], BF16)
            wqk = pp.tile([P, 4, KS, P], BF16)
            wv = pp.tile([P, KS, 2 * D2], BF16)
            wp = pp.tile([P, 2, T], BF16)
            qk = pp.tile([P, 4, T], BF16)  # m: q1|q2|k1|k2, [d, T] layout
            ident = pp.tile([P, P], BF16)
            vb = pp.tile([P, KS, 2, 130], BF16)  # [tk, kslab, vh, v|1|pad]

            nc.scalar.dma_start(out=xb[:, 0], in_=xb_d[:, 0])
            nc.scalar.dma_start(out=xb[:, 1], in_=xb_d[:, 1])
            nc.scalar.dma_start(out=xb[:, 2], in_=xb_d[:, 2])
            nc.scalar.dma_start(out=xb[:, 3], in_=xb_d[:, 3])
            nc.sync.dma_start(out=wqk[:, 2], in_=wqk_d[:, 2])  # k1 first
            nc.sync.dma_start(out=wqk[:, 3], in_=wqk_d[:, 3])
            nc.sync.dma_start(out=wv, in_=wv_d[:])
            nc.sync.dma_start(out=wqk[:, 0], in_=wqk_d[:, 0])
            nc.sync.dma_start(out=wqk[:, 1], in_=wqk_d[:, 1])
            nc.sync.dma_start(out=wp, in_=wp_d[:])
            nc.sync.dma_start(out=ident, in_=id_d[:])
            nc.gpsimd.memset(vb[:, :, :, D2:D2 + 1], 1.0)

            # ---------------- phase 1: qkv projections ----------------
            # (PSUM->SBUF copies ride the ACT engine, idle until the first
            # exp; GPSIMD/Pool cannot read PSUM)
            def emit_qkv_m(n, m):
                ps = scp.tile([P, NCH], F32, tag="s", name=f"psq{n}{m}")
                for k in range(KS):
                    nc.tensor.matmul(
                        ps,
                        wqk[:, m, k, :],
                        xb[:, n, k, :],
                        start=(k == 0),
                        stop=(k == KS - 1),
                    )
                nc.scalar.activation(qk[:, m, n * NCH:(n + 1) * NCH], ps, CPY)

            def emit_v(n, t2):
                g = 4 * n + t2
                ps = scp.tile([P, 2, D2], F32, tag="s", name=f"psv{g}")
                for k in range(KS):
                    nc.tensor.matmul(
                        ps,
                        xb[:, n, k, t2 * P:(t2 + 1) * P],
                        wv[:, k, :],
                        start=(k == 0),
                        stop=(k == KS - 1),
                    )
                nc.scalar.activation(vb[:, g, :, 0:D2], ps, CPY)

            for n in range(4):
                for m in (2, 3):  # k1, k2 (stationaries for all sweeps)
                    emit_qkv_m(n, m)
                for t2 in range(4):
                    emit_v(n, t2)
            for n in (0, 1):
                for m in (0, 1):  # q1, q2 for block 0
                    emit_qkv_m(n, m)

            # ---- filler machinery: PE work injected into ACT-bound sweeps
            q_fillers = []  # q projections for tq-block 1 (drain in block 0)
            tp_fillers = []  # deferred PE transposes (gated: DVE deps lag)
            proj_fillers = []  # block-0 output projection (drain in block 1)

            def inject(budget_ns, tp_ok=True):
                while budget_ns > 0:
                    if q_fillers:
                        lst = q_fillers
                    elif tp_fillers:
                        if not tp_ok:
                            return  # keep ordering: proj waits for tps
                        lst = tp_fillers
                    elif proj_fillers:
                        lst = proj_fillers
                    else:
                        return
                    est, f = lst.pop(0)
                    f()
                    budget_ns -= est

            def queue_q_fillers(n, m):
                # q projections for tq-block 1, chunk n, using the yp psum
                # slot (idle during block-0 sweeps)
                box = {}

                def mk(k):
                    def f():
                        if k == 0:
                            box["ps"] = ypp.tile(
                                [P, NCH], F32, tag="y", name=f"psq{n}{m}"
                            )
                        nc.tensor.matmul(
                            box["ps"],
                            wqk[:, m, k, :],
                            xb[:, n, k, :],
                            start=(k == 0),
                            stop=(k == KS - 1),
                        )
                        if k == KS - 1:
                            nc.vector.tensor_copy(
                                qk[:, m, n * NCH:(n + 1) * NCH], box["ps"]
                            )

                    return (220.0, f)

                q_fillers.extend(mk(k) for k in range(KS))

            for n in (2, 3):
                for m in (0, 1):
                    queue_q_fillers(n, m)

            def emit_tp(o2, vh, q, otT, psum_src=None, act_copy=False):
                pool, tag = psum_src if psum_src else (scp, "s")
                pt = pool.tile([P, P], BF16, tag=tag, name=f"tp{vh}{q}")
                nc.tensor.transpose(pt, o2[:, q, :], ident)
                if act_copy:
                    nc.scalar.activation(otT[:, vh, q, :], pt, CPY)
                else:
                    nc.vector.tensor_copy(otT[:, vh, q, :], pt)

            def queue_tp_pairs(o2, vh, otT):
                for q0 in range(0, QS, 2):
                    def f(q0=q0):
                        emit_tp(o2, vh, q0, otT)
                        emit_tp(o2, vh, q0 + 1, otT)
                    tp_fillers.append((120.0, f))

            # ---------------- phase 2: attention sweeps ----------------
            def get_accs(bk, vh, br):
                a = accp.tile([P, 3, 132], F32, tag="accA", name=f"accA{bk}{vh}{br}")
                b = accp.tile([P, 3, 132], F32, tag="accB", name=f"accB{bk}{vh}{br}")
                c = accp.tile([P, 2, 132], F32, tag="accC", name=f"accC{bk}{vh}{br}")
                return [(a, 0), (a, 1), (a, 2), (b, 0), (b, 1), (b, 2), (c, 0), (c, 1)]

            def sweep(bk, vh, br):
                # scores + exp + pv for head-pair (vh,br), tq block bk,
                # software-pipelined one k-slab ahead so the PE never waits
                # a full exp latency
                rows = slice(vh * H_DIM, (vh + 1) * H_DIM)
                accs = get_accs(bk, vh, br)
                ets = [None] * TT
                for j in range(TT + 1):
                    if j < TT:
                        ps = scp.tile(
                            [P, 2 * NCH], F32, tag="s", name=f"sc{bk}{vh}{br}{j}"
                        )
                        for i in range(2):
                            nc.tensor.matmul(
                                ps[:, i * NCH:(i + 1) * NCH],
                                qk[rows, 2 + br, j * P:(j + 1) * P],
                                qk[rows, br, bk * 1024 + i * NCH:
                                   bk * 1024 + (i + 1) * NCH],
                                start=True,
                                stop=True,
                            )
                        et = etp.tile(
                            [P, 2 * NCH], BF16, tag="e", name=f"et{bk}{vh}{br}{j}"
                        )
                        nc.scalar.activation(et, ps, EXP, scale=SCALE)
                        ets[j] = et
                    if j > 0:
                        inject(900.0 if j == 1 else 220.0, tp_ok=(j >= 8))
                        et = ets[j - 1]
                        for q in range(QS):
                            at, qi = accs[q]
                            # one psum accumulation group per BANK: start
                            # zeroes the whole 2KB zero-region lazily
                            first = (j - 1 == 0) and qi == 0
                            lastq = qi == (3 if q < 6 else 2) - 1
                            nc.tensor.matmul(
                                at[:, qi, 0:129],
                                et[:, q * P:(q + 1) * P],
                                vb[:, j - 1, vh, 0:129],
                                start=first,
                                stop=(j - 1 == TT - 1) and lastq,
                            )
                        ets[j - 1] = None
                return accs

            def save_accs(bk, vh, br, accs, tag):
                # copy a|r psum accumulators to SBUF (frees the acc banks
                # for the next sweep after ~3 DVE ops)
                sb = wkp.tile([P, QS, 132], F32, tag=tag, name=f"{tag}{bk}{vh}")
                nc.vector.tensor_copy(sb[:, 0:3, 0:129], accs[0][0][:, :, 0:129])
                nc.vector.tensor_copy(sb[:, 3:6, 0:129], accs[3][0][:, :, 0:129])
                nc.vector.tensor_copy(sb[:, 6:8, 0:129], accs[6][0][:, :, 0:129])
                return sb

            SQR = mybir.ActivationFunctionType.Square

            def combine(bk, vh, asb, bsb, otT, tail=False):
                # o' = a1*r2 - lam*a2*r1 (per-column rescale of the true o;
                # RMSNorm cancels it), then per-head RMS + bf16 + transpose.
                r1n = wkp.tile([P, QS, 1], F32, tag="r1n", name=f"r1n{bk}{vh}")
                o12 = wkp.tile([P, QS, P], F32, tag="o12", name=f"o12{bk}{vh}")
                sqs = wkp.tile([P, P], F32, tag="sqs", name=f"sqs{bk}{vh}")
                msb = wkp.tile([P, QS, 1], F32, tag="msb", name=f"msb{bk}{vh}")
                lns = wkp.tile([P, QS, 1], F32, tag="lns", name=f"lns{bk}{vh}")
                rs = wkp.tile([P, QS, 1], F32, tag="rs", name=f"rs{bk}{vh}")
                o2 = wkp.tile([P, QS, P], BF16, tag="o2", name=f"o2{bk}{vh}")
                nc.vector.tensor_scalar_mul(r1n, asb[:, :, 128:129], -lam)

                def rms_tail(qs):
                    nc.scalar.activation(
                        lns[:, qs, :], msb[:, qs, :], LOG, scale=1.0 / D2
                    )
                    nc.scalar.activation(
                        rs[:, qs, :], lns[:, qs, :], EXP, scale=-0.5
                    )

                for q in range(QS):
                    nc.vector.tensor_scalar_mul(
                        o12[:, q, :], asb[:, q, 0:P], bsb[:, q, 128:129]
                    )
                    nc.vector.scalar_tensor_tensor(
                        o12[:, q, :], bsb[:, q, 0:P], r1n[:, q, :], o12[:, q, :],
                        op0=MULT, op1=ADD,
                    )
                    if tail:
                        nc.scalar.activation(sqs, o12[:, q, :], SQR,
                                             accum_out=msb[:, q, :])
                    else:
                        nc.vector.tensor_mul(sqs, o12[:, q, :], o12[:, q, :])
                        nc.vector.tensor_reduce(
                            msb[:, q, :], sqs, mybir.AxisListType.X, ADD
                        )
                rms_tail(slice(0, QS))
                for q in range(QS):
                    if tail:
                        nc.scalar.activation(o2[:, q, :], o12[:, q, :], CPY,
                                             scale=rs[:, q, :])
                    else:
                        nc.vector.tensor_scalar_mul(
                            o2[:, q, :], o12[:, q, :], rs[:, q, :]
                        )
                queue_tp_pairs(o2, vh, otT)

            # -------- output projection for one 128-row tq tile ----------
            # psum_src: (pool, tag) for this tile's psum bank. Mid-kernel
            # (filler path) only the 1-bank "y" tag is free and copies go to
            # DVE; at the tail the attention accumulator banks are stolen
            # for a 4-bank rotation and copies split DVE/ACT.
            def proj_tile_closures(otT, t, psum_src, split_copies):
                q = t % QS
                pool, tag = psum_src
                box = {}
                cl = []

                def c_vh0(p):
                    def f():
                        if p == 0:
                            box["yp"] = pool.tile([P, 2, 256], F32, tag=tag,
                                                  name=f"yt{t}")
                            box["ys"] = ysp.tile([P, 4, 2, 256], BF16,
                                                 tag="ysb", name=f"ys{t}")
                        yp = box["yp"]
                        for r in range(2):
                            nc.tensor.matmul(
                                yp[:, r, :],
                                otT[:, 0, q, :],
                                wp[:, 0, 512 * p + 256 * r:512 * p + 256 * (r + 1)],
                                start=(r == 0),
                                stop=False,
                            )
                    return (230.0, f)

                def c_vh1(p):
                    def f():
                        yp = box["yp"]
                        for r in range(2):
                            nc.tensor.matmul(
                                yp[:, r, :],
                                otT[:, 1, q, :],
                                wp[:, 1, 512 * p + 256 * r:512 * p + 256 * (r + 1)],
                                start=False,
                                stop=(r == 1),
                            )
                        if split_copies:
                            nc.vector.tensor_copy(box["ys"][:, p, 0, :],
                                                  yp[:, 0, :])
                            nc.scalar.activation(box["ys"][:, p, 1, :],
                                                 yp[:, 1, :], CPY)
                        else:
                            nc.vector.tensor_copy(box["ys"][:, p, :, :], yp)
                    return (230.0, f)

                def c_dma():
                    nc.sync.dma_start(out=y_d[t], in_=box["ys"])

                for p in range(4):
                    cl.append(c_vh0(p))
                    cl.append(c_vh1(p))
                cl.append((0.0, c_dma))
                return cl

            # ---------------- blocks ----------------
            tail_rot = [(ypp, "y"), (accp, "accA"), (accp, "accB"),
                        (accp, "accC")]
            for bk in range(NBLK):
                if bk == 1:
                    # block-1 scores read the filler-produced q projections:
                    # force-drain any q fillers that block 0 didn't absorb
                    while q_fillers:
                        q_fillers.pop(0)[1]()
                otT = wkp.tile([P, 2, QS, P], BF16, tag="otT", name=f"otT{bk}")
                last = bk == NBLK - 1
                for vh in range(2):
                    accs0 = sweep(bk, vh, 0)
                    asb = save_accs(bk, vh, 0, accs0, "asb")
                    accs1 = sweep(bk, vh, 1)
                    bsb = save_accs(bk, vh, 1, accs1, "bsb")
                    combine(bk, vh, asb, bsb, otT, tail=(last and vh == 1))
                if not last:
                    for t in range(QS):
                        proj_fillers.extend(
                            proj_tile_closures(otT, bk * QS + t, (ypp, "y"),
                                               split_copies=False))
                else:
                    inject(1e9)  # drain leftovers (incl. final transposes)
                    for t in range(QS):
                        for est, f in proj_tile_closures(
                                otT, bk * QS + t, tail_rot[t % 4],
                                split_copies=True):
                            f()
    nc.finalize()
    return nc


def _core_inputs(x, w_qkv, w_proj, rms_scale):
    """Host-side shard prep: per-core bf16 weight slices + replicated x^T."""
    bf = mybir.dt.np(BF16)
    ident = np.ascontiguousarray(np.eye(P, dtype=np.float32).astype(bf))
    xt = x.reshape(T, C).T  # [C, T]
    xtr = np.ascontiguousarray(
        xt.reshape(KS, P, 4, NCH).transpose(1, 2, 0, 3).astype(bf)
    )
    sv = np.tile(
        rms_scale.astype(np.float32) * np.float32(1.0 - LAMBDA_INIT), 2
    )  # [256], per-row scale for this core's w_proj rows
    maps = []
    for c in range(N_CORES):
        cols = [
            w_qkv[:, 0 * 1024 + c * P:0 * 1024 + (c + 1) * P],  # q1 heads 2c,2c+1
            w_qkv[:, 1 * 1024 + c * P:1 * 1024 + (c + 1) * P],  # q2
            w_qkv[:, 2 * 1024 + c * P:2 * 1024 + (c + 1) * P],  # k1
            w_qkv[:, 3 * 1024 + c * P:3 * 1024 + (c + 1) * P],  # k2
        ]
        wqk = np.stack(cols, axis=0)  # [4, C, 128]
        wqk = np.ascontiguousarray(
            wqk.reshape(4, KS, P, P).transpose(2, 0, 1, 3).astype(bf)
        )
        wv = w_qkv[:, 2 * C + c * 2 * D2:2 * C + (c + 1) * 2 * D2]  # [C, 256]
        wv = np.ascontiguousarray(
            wv.reshape(KS, P, 2 * D2).transpose(1, 0, 2).astype(bf)
        )
        wp = w_proj[c * 2 * D2:(c + 1) * 2 * D2, :] * sv[:, None]  # [256, T]
        wp = np.ascontiguousarray(
            wp.reshape(2, P, T).transpose(1, 0, 2).astype(bf)
        )
        maps.append({"xt": xtr, "wqk": wqk, "wv": wv, "wp": wp, "ident": ident})
    return maps


def kernel(x, w_qkv, w_proj, lambda_q1, lambda_k1, lambda_q2, lambda_k2, rms_scale):
    from concourse.bass_utils import run_bass_kernel_spmd

    x = np.asarray(x, dtype=np.float32)
    w_qkv = np.asarray(w_qkv, dtype=np.float32)
    w_proj = np.asarray(w_proj, dtype=np.float32)
    rms_scale = np.asarray(rms_scale, dtype=np.float32)
    lam1 = np.exp(np.sum(np.asarray(lambda_q1) * np.asarray(lambda_k1), dtype=np.float32))
    lam2 = np.exp(np.sum(np.asarray(lambda_q2) * np.asarray(lambda_k2), dtype=np.float32))
    lam = float(lam1 - lam2 + LAMBDA_INIT)

    nc = build(lam)
    in_maps = _core_inputs(x, w_qkv, w_proj, rms_scale)
    res = run_bass_kernel_spmd(nc, in_maps, core_ids=list(range(N_CORES)))
    y = np.zeros((TT, P, T), np.float32)
    for rmap in res.results:
        y += np.asarray(rmap["y"], np.float32)
    return y.reshape(1, T, C)


# revision 23
# speedup vs baseline: 1.0431x; 1.0431x over previous
"""Trainium2 Bass kernel for DiffSelfAttention (B=1, T=2048, C=2048, 16 v-heads).

Sharding: tensor-parallel over heads across 8 NeuronCores. Core c owns
v-heads {2c, 2c+1} plus the matching q/k heads of both differential branches.
Each core computes its qkv slice, the attention for its 4 q/k head-pairs, the
differential + per-head RMSNorm, and a partial projection
y_c = out_c @ w_proj[rows_c]. The host sums the 8 partials (unshard step).

v2 design notes (vs the fp32r v1):
  - Everything bf16 on the PE (1 cycle/row at ANY moving size, halved DMA
    and SBUF footprint). Host converts inputs; rel-err budget is 2e-2 and
    bf16 keeps us ~1e-2 or better.
  - Transposed PV: attention is computed as a^T[tq,d2] = et^T @ [v|1] with
    the exp'd scores as the STATIONARY operand and [v | ones-column] as a
    129-wide moving operand. This gets the softmax denominator r in the
    same matmul (column 128) AND puts r on the partition axis, so all the
    differential-combine scalars are per-partition [P,1] operands — no
    broadcast matmuls. v1's separate ones-colsum (131k cycles/core) is gone.
  - Softmax divisions eliminated as in v1: RMSNorm is invariant to any
    per-column positive scale, so o' = a1*r2 - lam*a2*r1 feeds the norm.
  - RMS rsqrt = exp(-0.5*ln(mean)) on ACT, batched [P,8] per (block,head)
    to bound Ln/Exp table swaps (Ln and Exp are in different ACT table
    sets on this compiler: per-element ln/exp costs a 1.3us table load).
    rms_scale * (1-lambda_init) is folded into w_proj rows on the host.
  - o'[tq,d2] is transposed back to [d2,tq] for the projection on the PE
    (identity-matmul transpose). DmaTransposeAnt and tensor_tensor_reduce
    both kill the exec unit on this runtime - do not use them.
  - Work is emitted in 2 tq-blocks of 1024; the q projections for the
    second half, the deferred transposes, and the first block's output
    projection are injected as fillers into the (ACT-bound) attention
    sweeps so the PE never idles. Transposes are injected in PAIRS to
    keep the 2-slot scores-psum rotation aligned, and only from k-iter 8
    so their DVE deps are ready.
  - PSUM: one accumulation group per 2KB bank (start=True lazily zeroes
    the whole bank): scores [P,1024]x2 = 4 banks, pv accumulators
    (8 x [P,132] packed 3-per-bank, one group per bank) = 3, proj = 1.
    The tail projection steals the 3 freed accumulator banks.
  - Input DMAs split across the two HWDGE queues (SP: weights,
    ACT: x chunks) so the transfers overlap; ~12us to first matmul.
"""

import math

import numpy as np

import concourse.bass as bass
import concourse.bacc as bacc
import concourse.mybir as mybir
import concourse.tile as tile

F32 = mybir.dt.float32
BF16 = mybir.dt.bfloat16

T = 2048
C = 2048
N_HEAD = 16
H_DIM = 64
D2 = 2 * H_DIM  # 128 (v-head dim, also the RMS group size)
LAMBDA_INIT = 0.8 - 0.6 * math.exp(-0.3)
SCALE = 1.0 / math.sqrt(H_DIM)
P = 128
KS = C // P  # 16 contraction slabs
TT = T // P  # 16 t-tiles
NCH = 512  # phase-1 t-chunk width
QS = 8  # tq slabs per block
NBLK = 2  # tq blocks of 1024
N_CORES = 8

EXP = mybir.ActivationFunctionType.Exp
LOG = mybir.ActivationFunctionType.Ln
CPY = mybir.ActivationFunctionType.Copy
MULT = mybir.AluOpType.mult
ADD = mybir.AluOpType.add


def build(lam: float) -> bass.Bass:
    nc = bacc.Bacc("TRN2", target_bir_lowering=False, debug=False)

    xb_d = nc.dram_tensor("xt", [P, 4, KS, NCH], BF16, kind="ExternalInput")
    wqk_d = nc.dram_tensor("wqk", [P, 4, KS, P], BF16, kind="ExternalInput")
    wv_d = nc.dram_tensor("wv", [P, KS, 2 * D2], BF16, kind="ExternalInput")
    wp_d = nc.dram_tensor("wp", [P, 2, T], BF16, kind="ExternalInput")
    id_d = nc.dram_tensor("ident", [P, P], BF16, kind="ExternalInput")
    y_d = nc.dram_tensor("y", [TT, P, T], BF16, kind="ExternalOutput")

    with tile.TileContext(nc) as tc:
        with tc.tile_pool(name="persist", bufs=1) as pp, \
             tc.tile_pool(name="etp", bufs=3) as etp, \
             tc.tile_pool(name="work", bufs=2) as wkp, \
             tc.tile_pool(name="ysp", bufs=2) as ysp, \
             tc.tile_pool(name="sc", bufs=2, space="PSUM") as scp, \
             tc.tile_pool(name="acc", bufs=1, space="PSUM") as accp, \
             tc.tile_pool(name="yp", bufs=1, space="PSUM") as ypp:

            xb = pp.tile([P, 4, KS, NCH], BF16)
            wqk = pp.tile([P, 4, KS, P], BF16)
            wv = pp.tile([P, KS, 2 * D2], BF16)
            wp = pp.tile([P, 2, T], BF16)
            qk = pp.tile([P, 4, T], BF16)  # m: q1|q2|k1|k2, [d, T] layout
            ident = pp.tile([P, P], BF16)
            vb = pp.tile([P, KS, 2, 130], BF16)  # [tk, kslab, vh, v|1|pad]

            nc.scalar.dma_start(out=xb[:, 0], in_=xb_d[:, 0])
            nc.scalar.dma_start(out=xb[:, 1], in_=xb_d[:, 1])
            nc.scalar.dma_start(out=xb[:, 2], in_=xb_d[:, 2])
            nc.scalar.dma_start(out=xb[:, 3], in_=xb_d[:, 3])
            nc.sync.dma_start(out=wqk[:, 2], in_=wqk_d[:, 2])  # k1 first
            nc.sync.dma_start(out=wqk[:, 3], in_=wqk_d[:, 3])
            nc.sync.dma_start(out=wv, in_=wv_d[:])
            nc.sync.dma_start(out=wqk[:, 0], in_=wqk_d[:, 0])
            nc.sync.dma_start(out=wqk[:, 1], in_=wqk_d[:, 1])
            nc.sync.dma_start(out=wp, in_=wp_d[:])
            nc.sync.dma_start(out=ident, in_=id_d[:])
            nc.gpsimd.memset(vb[:, :, :, D2:D2 + 1], 1.0)

            # ---------------- phase 1: qkv projections ----------------
            # (PSUM->SBUF copies ride the ACT engine, idle until the first
            # exp; GPSIMD/Pool cannot read PSUM)
            def emit_qkv_m(n, m):
                ps = scp.tile([P, NCH], F32, tag="s", name=f"psq{n}{m}")
                for k in range(KS):
                    nc.tensor.matmul(
                        ps,
                        wqk[:, m, k, :],
                        xb[:, n, k, :],
                        start=(k == 0),
                        stop=(k == KS - 1),
                    )
                nc.scalar.activation(qk[:, m, n * NCH:(n + 1) * NCH], ps, CPY)

            def emit_v(n, t2):
                g = 4 * n + t2
                ps = scp.tile([P, 2, D2], F32, tag="s", name=f"psv{g}")
                for k in range(KS):
                    nc.tensor.matmul(
                        ps,
                        xb[:, n, k, t2 * P:(t2 + 1) * P],
                        wv[:, k, :],
                        start=(k == 0),
                        stop=(k == KS - 1),
                    )
                nc.scalar.activation(vb[:, g, :, 0:D2], ps, CPY)

            for n in range(4):
                for m in (2, 3):  # k1, k2 (stationaries for all sweeps)
                    emit_qkv_m(n, m)
                for t2 in range(4):
                    emit_v(n, t2)
            for n in (0, 1):
                for m in (0, 1):  # q1, q2 for block 0
                    emit_qkv_m(n, m)

            # ---- filler machinery: PE work injected into ACT-bound sweeps
            q_fillers = []  # q projections for tq-block 1 (drain in block 0)
            tp_fillers = []  # deferred PE transposes (gated: DVE deps lag)
            proj_fillers = []  # block-0 output projection (drain in block 1)

            def inject(budget_ns, tp_ok=True):
                while budget_ns > 0:
                    if q_fillers:
                        lst = q_fillers
                    elif tp_fillers:
                        if not tp_ok:
                            return  # keep ordering: proj waits for tps
                        lst = tp_fillers
                    elif proj_fillers:
                        lst = proj_fillers
                    else:
                        return
                    est, f = lst.pop(0)
                    f()
                    budget_ns -= est

            def queue_q_fillers(n, m):
                # q projections for tq-block 1, chunk n, using the yp psum
                # slot (idle during block-0 sweeps)
                box = {}

                def mk(k):
                    def f():
                        if k == 0:
                            box["ps"] = ypp.tile(
                                [P, NCH], F32, tag="y", name=f"psq{n}{m}"
                            )
                        nc.tensor.matmul(
                            box["ps"],
                            wqk[:, m, k, :],
                            xb[:, n, k, :],
                            start=(k == 0),
                            stop=(k == KS - 1),
                        )
                        if k == KS - 1:
                            nc.vector.tensor_copy(
                                qk[:, m, n * NCH:(n + 1) * NCH], box["ps"]
                            )

                    return (220.0, f)

                q_fillers.extend(mk(k) for k in range(KS))

            for n in (2, 3):
                for m in (0, 1):
                    queue_q_fillers(n, m)

            def emit_tp(o2, vh, q, otT, psum_src=None, act_copy=False):
                pool, tag = psum_src if psum_src else (scp, "s")
                pt = pool.tile([P, P], BF16, tag=tag, name=f"tp{vh}{q}")
                nc.tensor.transpose(pt, o2[:, q, :], ident)
                if act_copy:
                    nc.scalar.activation(otT[:, vh, q, :], pt, CPY)
                else:
                    nc.vector.tensor_copy(otT[:, vh, q, :], pt)

            def queue_tp_pairs(o2, vh, otT):
                for q0 in range(0, QS, 2):
                    def f(q0=q0):
                        emit_tp(o2, vh, q0, otT)
                        emit_tp(o2, vh, q0 + 1, otT)
                    tp_fillers.append((120.0, f))

            # ---------------- phase 2: attention sweeps ----------------
            def get_accs(bk, vh, br):
                a = accp.tile([P, 3, 132], F32, tag="accA", name=f"accA{bk}{vh}{br}")
                b = accp.tile([P, 3, 132], F32, tag="accB", name=f"accB{bk}{vh}{br}")
                c = accp.tile([P, 2, 132], F32, tag="accC", name=f"accC{bk}{vh}{br}")
                return [(a, 0), (a, 1), (a, 2), (b, 0), (b, 1), (b, 2), (c, 0), (c, 1)]

            def sweep(bk, vh, br):
                # scores + exp + pv for head-pair (vh,br), tq block bk,
                # software-pipelined one k-slab ahead so the PE never waits
                # a full exp latency
                rows = slice(vh * H_DIM, (vh + 1) * H_DIM)
                accs = get_accs(bk, vh, br)
                ets = [None] * TT
                for j in range(TT + 1):
                    if j < TT:
                        ps = scp.tile(
                            [P, 2 * NCH], F32, tag="s", name=f"sc{bk}{vh}{br}{j}"
                        )
                        for i in range(2):
                            nc.tensor.matmul(
                                ps[:, i * NCH:(i + 1) * NCH],
                                qk[rows, 2 + br, j * P:(j + 1) * P],
                                qk[rows, br, bk * 1024 + i * NCH:
                                   bk * 1024 + (i + 1) * NCH],
                                start=True,
                                stop=True,
                            )
                        et = etp.tile(
                            [P, 2 * NCH], BF16, tag="e", name=f"et{bk}{vh}{br}{j}"
                        )
                        nc.scalar.activation(et, ps, EXP, scale=SCALE)
                        ets[j] = et
                    if j > 0:
                        inject(900.0 if j == 1 else 220.0, tp_ok=(j >= 8))
                        et = ets[j - 1]
                        for q in range(QS):
                            at, qi = accs[q]
                            # one psum accumulation group per BANK: start
                            # zeroes the whole 2KB zero-region lazily
                            first = (j - 1 == 0) and qi == 0
                            lastq = qi == (3 if q < 6 else 2) - 1
                            nc.tensor.matmul(
                                at[:, qi, 0:129],
                                et[:, q * P:(q + 1) * P],
                                vb[:, j - 1, vh, 0:129],
                                start=first,
                                stop=(j - 1 == TT - 1) and lastq,
                            )
                        ets[j - 1] = None
                return accs

            def save_accs(bk, vh, br, accs, tag):
                # copy a|r psum accumulators to SBUF (frees the acc banks
                # for the next sweep after ~3 DVE ops)
                sb = wkp.tile([P, QS, 132], F32, tag=tag, name=f"{tag}{bk}{vh}")
                nc.vector.tensor_copy(sb[:, 0:3, 0:129], accs[0][0][:, :, 0:129])
                nc.vector.tensor_copy(sb[:, 3:6, 0:129], accs[3][0][:, :, 0:129])
                nc.vector.tensor_copy(sb[:, 6:8, 0:129], accs[6][0][:, :, 0:129])
                return sb

            SQR = mybir.ActivationFunctionType.Square

            def combine(bk, vh, asb, bsb, otT, tail=False):
                # o' = a1*r2 - lam*a2*r1 (per-column rescale of the true o;
                # RMSNorm cancels it), then per-head RMS + bf16 + transpose.
                r1n = wkp.tile([P, QS, 1], F32, tag="r1n", name=f"r1n{bk}{vh}")
                o12 = wkp.tile([P, QS, P], F32, tag="o12", name=f"o12{bk}{vh}")
                sqs = wkp.tile([P, P], F32, tag="sqs", name=f"sqs{bk}{vh}")
                msb = wkp.tile([P, QS, 1], F32, tag="msb", name=f"msb{bk}{vh}")
                lns = wkp.tile([P, QS, 1], F32, tag="lns", name=f"lns{bk}{vh}")
                rs = wkp.tile([P, QS, 1], F32, tag="rs", name=f"rs{bk}{vh}")
                o2 = wkp.tile([P, QS, P], BF16, tag="o2", name=f"o2{bk}{vh}")
                nc.vector.tensor_scalar_mul(r1n, asb[:, :, 128:129], -lam)

                def rms_tail(qs):
                    nc.scalar.activation(
                        lns[:, qs, :], msb[:, qs, :], LOG, scale=1.0 / D2
                    )
                    nc.scalar.activation(
                        rs[:, qs, :], lns[:, qs, :], EXP, scale=-0.5
                    )

                for q in range(QS):
                    nc.vector.tensor_scalar_mul(
                        o12[:, q, :], asb[:, q, 0:P], bsb[:, q, 128:129]
                    )
                    nc.vector.scalar_tensor_tensor(
                        o12[:, q, :], bsb[:, q, 0:P], r1n[:, q, :], o12[:, q, :],
                        op0=MULT, op1=ADD,
                    )
                    if tail:
                        nc.scalar.activation(sqs, o12[:, q, :], SQR,
                                             accum_out=msb[:, q, :])
                    else:
                        nc.vector.tensor_mul(sqs, o12[:, q, :], o12[:, q, :])
                        nc.vector.tensor_reduce(
                            msb[:, q, :], sqs, mybir.AxisListType.X, ADD
                        )
                rms_tail(slice(0, QS))
                for q in range(QS):
                    if tail:
                        nc.scalar.activation(o2[:, q, :], o12[:, q, :], CPY,
                                             scale=rs[:, q, :])
                    else:
                        nc.vector.tensor_scalar_mul(
                            o2[:, q, :], o12[:, q, :], rs[:, q, :]
                        )
                queue_tp_pairs(o2, vh, otT)

            # -------- output projection for one 128-row tq tile ----------
            # psum_src: (pool, tag) for this tile's psum bank. Mid-kernel
            # (filler path) only the 1-bank "y" tag is free and copies go to
            # DVE; at the tail the attention accumulator banks are stolen
            # for a 4-bank rotation and copies split DVE/ACT.
            def proj_tile_closures(otT, t, psum_src, split_copies):
                q = t % QS
                pool, tag = psum_src
                box = {}
                cl = []

                def c_vh0(p):
                    def f():
                        if p == 0:
                            box["yp"] = pool.tile([P, 2, 256], F32, tag=tag,
                                                  name=f"yt{t}")
                            box["ys"] = ysp.tile([P, 4, 2, 256], BF16,
                                                 tag="ysb", name=f"ys{t}")
                        yp = box["yp"]
                        for r in range(2):
                            nc.tensor.matmul(
                                yp[:, r, :],
                                otT[:, 0, q, :],
                                wp[:, 0, 512 * p + 256 * r:512 * p + 256 * (r + 1)],
                                start=(r == 0),
                                stop=False,
                            )
                    return (230.0, f)

                def c_vh1(p):
                    def f():
                        yp = box["yp"]
                        for r in range(2):
                            nc.tensor.matmul(
                                yp[:, r, :],
                                otT[:, 1, q, :],
                                wp[:, 1, 512 * p + 256 * r:512 * p + 256 * (r + 1)],
                                start=False,
                                stop=(r == 1),
                            )
                        if split_copies:
                            nc.vector.tensor_copy(box["ys"][:, p, 0, :],
                                                  yp[:, 0, :])
                            nc.scalar.activation(box["ys"][:, p, 1, :],
                                                 yp[:, 1, :], CPY)
                        else:
                            nc.vector.tensor_copy(box["ys"][:, p, :, :], yp)
                    return (230.0, f)

                def c_dma():
                    nc.sync.dma_start(out=y_d[t], in_=box["ys"])

                for p in range(4):
                    cl.append(c_vh0(p))
                    cl.append(c_vh1(p))
                cl.append((0.0, c_dma))
                return cl

            # ---------------- blocks ----------------
            tail_rot = [(ypp, "y"), (accp, "accA"), (accp, "accB"),
                        (accp, "accC")]
            for bk in range(NBLK):
                if bk == 1:
                    # block-1 scores read the filler-produced q projections:
                    # force-drain any q fillers that block 0 didn't absorb
                    while q_fillers:
                        q_fillers.pop(0)[1]()
                otT = wkp.tile([P, 2, QS, P], BF16, tag="otT", name=f"otT{bk}")
                last = bk == NBLK - 1
                for vh in range(2):
                    accs0 = sweep(bk, vh, 0)
                    asb = save_accs(bk, vh, 0, accs0, "asb")
                    accs1 = sweep(bk, vh, 1)
                    bsb = save_accs(bk, vh, 1, accs1, "bsb")
                    combine(bk, vh, asb, bsb, otT, tail=(last and vh == 1))
                if not last:
                    for t in range(QS):
                        proj_fillers.extend(
                            proj_tile_closures(otT, bk * QS + t, (ypp, "y"),
                                               split_copies=False))
                else:
                    inject(1e9)  # drain leftovers (incl. final transposes)
                    for t in range(QS):
                        for est, f in proj_tile_closures(
                                otT, bk * QS + t, tail_rot[t % 4],
                                split_copies=True):
                            f()
    nc.finalize()
    return nc


def _core_inputs(x, w_qkv, w_proj, rms_scale):
    """Host-side shard prep: per-core bf16 weight slices + replicated x^T."""
    bf = mybir.dt.np(BF16)
    ident = np.ascontiguousarray(np.eye(P, dtype=np.float32).astype(bf))
    xt = x.reshape(T, C).T  # [C, T]
    xtr = np.ascontiguousarray(
        xt.reshape(KS, P, 4, NCH).transpose(1, 2, 0, 3).astype(bf)
    )
    sv = np.tile(
        rms_scale.astype(np.float32) * np.float32(1.0 - LAMBDA_INIT), 2
    )  # [256], per-row scale for this core's w_proj rows
    maps = []
    for c in range(N_CORES):
        cols = [
            w_qkv[:, 0 * 1024 + c * P:0 * 1024 + (c + 1) * P],  # q1 heads 2c,2c+1
            w_qkv[:, 1 * 1024 + c * P:1 * 1024 + (c + 1) * P],  # q2
            w_qkv[:, 2 * 1024 + c * P:2 * 1024 + (c + 1) * P],  # k1
            w_qkv[:, 3 * 1024 + c * P:3 * 1024 + (c + 1) * P],  # k2
        ]
        wqk = np.stack(cols, axis=0)  # [4, C, 128]
        wqk = np.ascontiguousarray(
            wqk.reshape(4, KS, P, P).transpose(2, 0, 1, 3).astype(bf)
        )
        wv = w_qkv[:, 2 * C + c * 2 * D2:2 * C + (c + 1) * 2 * D2]  # [C, 256]
        wv = np.ascontiguousarray(
            wv.reshape(KS, P, 2 * D2).transpose(1, 0, 2).astype(bf)
        )
        wp = w_proj[c * 2 * D2:(c + 1) * 2 * D2, :] * sv[:, None]  # [256, T]
        wp = np.ascontiguousarray(
            wp.reshape(2, P, T).transpose(1, 0, 2).astype(bf)
        )
        maps.append({"xt": xtr, "wqk": wqk, "wv": wv, "wp": wp, "ident": ident})
    return maps


def kernel(x, w_qkv, w_proj, lambda_q1, lambda_k1, lambda_q2, lambda_k2, rms_scale):
    from concourse.bass_utils import run_bass_kernel_spmd

    x = np.asarray(x, dtype=np.float32)
    w_qkv = np.asarray(w_qkv, dtype=np.float32)
    w_proj = np.asarray(w_proj, dtype=np.float32)
    rms_scale = np.asarray(rms_scale, dtype=np.float32)
    lam1 = np.exp(np.sum(np.asarray(lambda_q1) * np.asarray(lambda_k1), dtype=np.float32))
    lam2 = np.exp(np.sum(np.asarray(lambda_q2) * np.asarray(lambda_k2), dtype=np.float32))
    lam = float(lam1 - lam2 + LAMBDA_INIT)

    nc = build(lam)
    in_maps = _core_inputs(x, w_qkv, w_proj, rms_scale)
    res = run_bass_kernel_spmd(nc, in_maps, core_ids=list(range(N_CORES)))
    y = np.zeros((TT, P, T), np.float32)
    for rmap in res.results:
        y += np.asarray(rmap["y"], np.float32)
    return y.reshape(1, T, C)


# revision 24
# speedup vs baseline: 1.0802x; 1.0355x over previous
"""Trainium2 Bass kernel for DiffSelfAttention (B=1, T=2048, C=2048, 16 v-heads).

Sharding: tensor-parallel over heads across 8 NeuronCores. Core c owns
v-heads {2c, 2c+1} plus the matching q/k heads of both differential branches.
Each core computes its qkv slice, the attention for its 4 q/k head-pairs, the
differential + per-head RMSNorm, and a partial projection
y_c = out_c @ w_proj[rows_c]. The host sums the 8 partials (unshard step).

v2 design notes (vs the fp32r v1):
  - Everything bf16 on the PE (1 cycle/row at ANY moving size, halved DMA
    and SBUF footprint). Host converts inputs; rel-err budget is 2e-2 and
    bf16 keeps us ~1e-2 or better.
  - Transposed PV: attention is computed as a^T[tq,d2] = et^T @ [v|1] with
    the exp'd scores as the STATIONARY operand and [v | ones-column] as a
    129-wide moving operand. This gets the softmax denominator r in the
    same matmul (column 128) AND puts r on the partition axis, so all the
    differential-combine scalars are per-partition [P,1] operands — no
    broadcast matmuls. v1's separate ones-colsum (131k cycles/core) is gone.
  - Softmax divisions eliminated as in v1: RMSNorm is invariant to any
    per-column positive scale, so o' = a1*r2 - lam*a2*r1 feeds the norm.
  - RMS rsqrt = exp(-0.5*ln(mean)) on ACT, batched [P,8] per (block,head)
    to bound Ln/Exp table swaps (Ln and Exp are in different ACT table
    sets on this compiler: per-element ln/exp costs a 1.3us table load).
    rms_scale * (1-lambda_init) is folded into w_proj rows on the host.
  - o'[tq,d2] is transposed back to [d2,tq] for the projection on the PE
    (identity-matmul transpose). DmaTransposeAnt and tensor_tensor_reduce
    both kill the exec unit on this runtime - do not use them.
  - Work is emitted in 2 tq-blocks of 1024; the q projections for the
    second half, the deferred transposes, and the first block's output
    projection are injected as fillers into the (ACT-bound) attention
    sweeps so the PE never idles. Transposes are injected in PAIRS to
    keep the 2-slot scores-psum rotation aligned, and only from k-iter 8
    so their DVE deps are ready.
  - PSUM: one accumulation group per 2KB bank (start=True lazily zeroes
    the whole bank): scores [P,1024]x2 = 4 banks, pv accumulators
    (8 x [P,132] packed 3-per-bank, one group per bank) = 3, proj = 1.
    The tail projection steals the 3 freed accumulator banks.
  - Input DMAs split across the two HWDGE queues (SP: weights,
    ACT: x chunks) so the transfers overlap; ~12us to first matmul.
"""

import math

import numpy as np

import concourse.bass as bass
import concourse.bacc as bacc
import concourse.mybir as mybir
import concourse.tile as tile

F32 = mybir.dt.float32
BF16 = mybir.dt.bfloat16

T = 2048
C = 2048
N_HEAD = 16
H_DIM = 64
D2 = 2 * H_DIM  # 128 (v-head dim, also the RMS group size)
LAMBDA_INIT = 0.8 - 0.6 * math.exp(-0.3)
SCALE = 1.0 / math.sqrt(H_DIM)
P = 128
KS = C // P  # 16 contraction slabs
TT = T // P  # 16 t-tiles
NCH = 512  # phase-1 t-chunk width
QS = 8  # tq slabs per block
NBLK = 2  # tq blocks of 1024
N_CORES = 8

EXP = mybir.ActivationFunctionType.Exp
LOG = mybir.ActivationFunctionType.Ln
CPY = mybir.ActivationFunctionType.Copy
MULT = mybir.AluOpType.mult
ADD = mybir.AluOpType.add


def build(lam: float) -> bass.Bass:
    nc = bacc.Bacc("TRN2", target_bir_lowering=False, debug=False)

    xb_d = nc.dram_tensor("xt", [P, 4, KS, NCH], BF16, kind="ExternalInput")
    wqk_d = nc.dram_tensor("wqk", [P, 4, KS, P], BF16, kind="ExternalInput")
    wv_d = nc.dram_tensor("wv", [P, KS, 2 * D2], BF16, kind="ExternalInput")
    wp_d = nc.dram_tensor("wp", [P, 2, T], BF16, kind="ExternalInput")
    id_d = nc.dram_tensor("ident", [P, P], BF16, kind="ExternalInput")
    y_d = nc.dram_tensor("y", [TT, P, T], BF16, kind="ExternalOutput")

    with tile.TileContext(nc) as tc:
        with tc.tile_pool(name="persist", bufs=1) as pp, \
             tc.tile_pool(name="etp", bufs=3) as etp, \
             tc.tile_pool(name="work", bufs=2) as wkp, \
             tc.tile_pool(name="ysp", bufs=2) as ysp, \
             tc.tile_pool(name="sc", bufs=2, space="PSUM") as scp, \
             tc.tile_pool(name="acc", bufs=1, space="PSUM") as accp, \
             tc.tile_pool(name="yp", bufs=1, space="PSUM") as ypp:

            xb = pp.tile([P, 4, KS, NCH], BF16)
            wqk = pp.tile([P, 4, KS, P], BF16)
            wv = pp.tile([P, KS, 2 * D2], BF16)
            wp = pp.tile([P, 2, T], BF16)
            qk = pp.tile([P, 4, T], BF16)  # m: q1|q2|k1|k2, [d, T] layout
            ident = pp.tile([P, P], BF16)
            vb = pp.tile([P, KS, 2, 130], BF16)  # [tk, kslab, vh, v|1|pad]

            nc.scalar.dma_start(out=xb[:, 0, 0:8], in_=xb_d[:, 0, 0:8])
            nc.scalar.dma_start(out=xb[:, 0, 8:16], in_=xb_d[:, 0, 8:16])
            nc.scalar.dma_start(out=xb[:, 2], in_=xb_d[:, 2])
            nc.sync.dma_start(out=wqk[:, 2], in_=wqk_d[:, 2])  # k1 first
            nc.sync.dma_start(out=wqk[:, 3], in_=wqk_d[:, 3])
            nc.sync.dma_start(out=xb[:, 1], in_=xb_d[:, 1])
            nc.sync.dma_start(out=wv, in_=wv_d[:])
            nc.sync.dma_start(out=wqk[:, 0], in_=wqk_d[:, 0])
            nc.sync.dma_start(out=wqk[:, 1], in_=wqk_d[:, 1])
            nc.sync.dma_start(out=xb[:, 3], in_=xb_d[:, 3])
            nc.sync.dma_start(out=wp, in_=wp_d[:])
            nc.sync.dma_start(out=ident, in_=id_d[:])
            nc.gpsimd.memset(vb[:, :, :, D2:D2 + 1], 1.0)

            # ---------------- phase 1: qkv projections ----------------
            # (PSUM->SBUF copies ride the ACT engine, idle until the first
            # exp; GPSIMD/Pool cannot read PSUM)
            def emit_qkv_m(n, m):
                ps = scp.tile([P, NCH], F32, tag="s", name=f"psq{n}{m}")
                for k in range(KS):
                    nc.tensor.matmul(
                        ps,
                        wqk[:, m, k, :],
                        xb[:, n, k, :],
                        start=(k == 0),
                        stop=(k == KS - 1),
                    )
                nc.scalar.activation(qk[:, m, n * NCH:(n + 1) * NCH], ps, CPY)

            def emit_v(n, t2):
                g = 4 * n + t2
                ps = scp.tile([P, 2, D2], F32, tag="s", name=f"psv{g}")
                for k in range(KS):
                    nc.tensor.matmul(
                        ps,
                        xb[:, n, k, t2 * P:(t2 + 1) * P],
                        wv[:, k, :],
                        start=(k == 0),
                        stop=(k == KS - 1),
                    )
                nc.scalar.activation(vb[:, g, :, 0:D2], ps, CPY)

            for n in range(4):
                for m in (2, 3):  # k1, k2 (stationaries for all sweeps)
                    emit_qkv_m(n, m)
                for t2 in range(4):
                    emit_v(n, t2)
            for n in (0, 1):
                for m in (0, 1):  # q1, q2 for block 0
                    emit_qkv_m(n, m)

            # ---- filler machinery: PE work injected into ACT-bound sweeps
            q_fillers = []  # q projections for tq-block 1 (drain in block 0)
            tp_fillers = []  # deferred PE transposes (gated: DVE deps lag)
            proj_fillers = []  # block-0 output projection (drain in block 1)

            def inject(budget_ns, tp_ok=True):
                while budget_ns > 0:
                    if q_fillers:
                        lst = q_fillers
                    elif tp_fillers:
                        if not tp_ok:
                            return  # keep ordering: proj waits for tps
                        lst = tp_fillers
                    elif proj_fillers:
                        lst = proj_fillers
                    else:
                        return
                    est, f = lst.pop(0)
                    f()
                    budget_ns -= est

            def queue_q_fillers(n, m):
                # q projections for tq-block 1, chunk n, using the yp psum
                # slot (idle during block-0 sweeps)
                box = {}

                def mk(k):
                    def f():
                        if k == 0:
                            box["ps"] = ypp.tile(
                                [P, NCH], F32, tag="y", name=f"psq{n}{m}"
                            )
                        nc.tensor.matmul(
                            box["ps"],
                            wqk[:, m, k, :],
                            xb[:, n, k, :],
                            start=(k == 0),
                            stop=(k == KS - 1),
                        )
                        if k == KS - 1:
                            nc.vector.tensor_copy(
                                qk[:, m, n * NCH:(n + 1) * NCH], box["ps"]
                            )

                    return (220.0, f)

                q_fillers.extend(mk(k) for k in range(KS))

            for n in (2, 3):
                for m in (0, 1):
                    queue_q_fillers(n, m)

            def emit_tp(o2, vh, q, otT, psum_src=None, act_copy=False):
                pool, tag = psum_src if psum_src else (scp, "s")
                pt = pool.tile([P, P], BF16, tag=tag, name=f"tp{vh}{q}")
                nc.tensor.transpose(pt, o2[:, q, :], ident)
                if act_copy:
                    nc.scalar.activation(otT[:, vh, q, :], pt, CPY)
                else:
                    nc.vector.tensor_copy(otT[:, vh, q, :], pt)

            def queue_tp_pairs(o2, vh, otT):
                for q0 in range(0, QS, 2):
                    def f(q0=q0):
                        emit_tp(o2, vh, q0, otT)
                        emit_tp(o2, vh, q0 + 1, otT)
                    tp_fillers.append((120.0, f))

            # ---------------- phase 2: attention sweeps ----------------
            def get_accs(bk, vh, br):
                a = accp.tile([P, 3, 132], F32, tag="accA", name=f"accA{bk}{vh}{br}")
                b = accp.tile([P, 3, 132], F32, tag="accB", name=f"accB{bk}{vh}{br}")
                c = accp.tile([P, 2, 132], F32, tag="accC", name=f"accC{bk}{vh}{br}")
                return [(a, 0), (a, 1), (a, 2), (b, 0), (b, 1), (b, 2), (c, 0), (c, 1)]

            def sweep(bk, vh, br):
                # scores + exp + pv for head-pair (vh,br), tq block bk,
                # software-pipelined one k-slab ahead so the PE never waits
                # a full exp latency
                rows = slice(vh * H_DIM, (vh + 1) * H_DIM)
                accs = get_accs(bk, vh, br)
                ets = [None] * TT
                for j in range(TT + 1):
                    if j < TT:
                        ps = scp.tile(
                            [P, 2 * NCH], F32, tag="s", name=f"sc{bk}{vh}{br}{j}"
                        )
                        for i in range(2):
                            nc.tensor.matmul(
                                ps[:, i * NCH:(i + 1) * NCH],
                                qk[rows, 2 + br, j * P:(j + 1) * P],
                                qk[rows, br, bk * 1024 + i * NCH:
                                   bk * 1024 + (i + 1) * NCH],
                                start=True,
                                stop=True,
                            )
                        et = etp.tile(
                            [P, 2 * NCH], BF16, tag="e", name=f"et{bk}{vh}{br}{j}"
                        )
                        nc.scalar.activation(et, ps, EXP, scale=SCALE)
                        ets[j] = et
                    if j > 0:
                        inject(900.0 if j == 1 else 220.0, tp_ok=(j >= 8))
                        et = ets[j - 1]
                        for q in range(QS):
                            at, qi = accs[q]
                            # one psum accumulation group per BANK: start
                            # zeroes the whole 2KB zero-region lazily
                            first = (j - 1 == 0) and qi == 0
                            lastq = qi == (3 if q < 6 else 2) - 1
                            nc.tensor.matmul(
                                at[:, qi, 0:129],
                                et[:, q * P:(q + 1) * P],
                                vb[:, j - 1, vh, 0:129],
                                start=first,
                                stop=(j - 1 == TT - 1) and lastq,
                            )
                        ets[j - 1] = None
                return accs

            def save_accs(bk, vh, br, accs, tag):
                # copy a|r psum accumulators to SBUF (frees the acc banks
                # for the next sweep after ~3 DVE ops)
                sb = wkp.tile([P, QS, 132], F32, tag=tag, name=f"{tag}{bk}{vh}")
                nc.vector.tensor_copy(sb[:, 0:3, 0:129], accs[0][0][:, :, 0:129])
                nc.scalar.activation(sb[:, 3:6, 0:129], accs[3][0][:, :, 0:129],
                                     CPY)
                nc.vector.tensor_copy(sb[:, 6:8, 0:129], accs[6][0][:, :, 0:129])
                return sb

            SQR = mybir.ActivationFunctionType.Square

            def combine(bk, vh, asb, bsb, otT, tail=False):
                # o' = a1*r2 - lam*a2*r1 (per-column rescale of the true o;
                # RMSNorm cancels it), then per-head RMS + bf16 + transpose.
                r1n = wkp.tile([P, QS, 1], F32, tag="r1n", name=f"r1n{bk}{vh}")
                o12 = wkp.tile([P, QS, P], F32, tag="o12", name=f"o12{bk}{vh}")
                sqs = wkp.tile([P, P], F32, tag="sqs", name=f"sqs{bk}{vh}")
                msb = wkp.tile([P, QS, 1], F32, tag="msb", name=f"msb{bk}{vh}")
                lns = wkp.tile([P, QS, 1], F32, tag="lns", name=f"lns{bk}{vh}")
                rs = wkp.tile([P, QS, 1], F32, tag="rs", name=f"rs{bk}{vh}")
                o2 = wkp.tile([P, QS, P], BF16, tag="o2", name=f"o2{bk}{vh}")
                nc.vector.tensor_scalar_mul(r1n, asb[:, :, 128:129], -lam)

                def rms_tail(qs):
                    nc.scalar.activation(
                        lns[:, qs, :], msb[:, qs, :], LOG, scale=1.0 / D2
                    )
                    nc.scalar.activation(
                        rs[:, qs, :], lns[:, qs, :], EXP, scale=-0.5
                    )

                for q in range(QS):
                    nc.vector.tensor_scalar_mul(
                        o12[:, q, :], asb[:, q, 0:P], bsb[:, q, 128:129]
                    )
                    nc.vector.scalar_tensor_tensor(
                        o12[:, q, :], bsb[:, q, 0:P], r1n[:, q, :], o12[:, q, :],
                        op0=MULT, op1=ADD,
                    )
                    if tail:
                        nc.scalar.activation(sqs, o12[:, q, :], SQR,
                                             accum_out=msb[:, q, :])
                    else:
                        nc.vector.tensor_mul(sqs, o12[:, q, :], o12[:, q, :])
                        nc.vector.tensor_reduce(
                            msb[:, q, :], sqs, mybir.AxisListType.X, ADD
                        )
                rms_tail(slice(0, QS))
                for q in range(QS):
                    if tail:
                        nc.scalar.activation(o2[:, q, :], o12[:, q, :], CPY,
                                             scale=rs[:, q, :])
                    else:
                        nc.vector.tensor_scalar_mul(
                            o2[:, q, :], o12[:, q, :], rs[:, q, :]
                        )
                queue_tp_pairs(o2, vh, otT)

            # -------- output projection for one 128-row tq tile ----------
            # psum_src: (pool, tag) for this tile's psum bank. Mid-kernel
            # (filler path) only the 1-bank "y" tag is free and copies go to
            # DVE; at the tail the attention accumulator banks are stolen
            # for a 4-bank rotation and copies split DVE/ACT.
            def proj_tile_closures(otT, t, psum_src, split_copies):
                q = t % QS
                pool, tag = psum_src
                box = {}
                cl = []

                def c_vh0(p):
                    def f():
                        if p == 0:
                            box["yp"] = pool.tile([P, 2, 256], F32, tag=tag,
                                                  name=f"yt{t}")
                            box["ys"] = ysp.tile([P, 4, 2, 256], BF16,
                                                 tag="ysb", name=f"ys{t}")
                        yp = box["yp"]
                        for r in range(2):
                            nc.tensor.matmul(
                                yp[:, r, :],
                                otT[:, 0, q, :],
                                wp[:, 0, 512 * p + 256 * r:512 * p + 256 * (r + 1)],
                                start=(r == 0),
                                stop=False,
                            )
                    return (230.0, f)

                def c_vh1(p):
                    def f():
                        yp = box["yp"]
                        for r in range(2):
                            nc.tensor.matmul(
                                yp[:, r, :],
                                otT[:, 1, q, :],
                                wp[:, 1, 512 * p + 256 * r:512 * p + 256 * (r + 1)],
                                start=False,
                                stop=(r == 1),
                            )
                        if split_copies:
                            nc.vector.tensor_copy(box["ys"][:, p, 0, :],
                                                  yp[:, 0, :])
                            nc.scalar.activation(box["ys"][:, p, 1, :],
                                                 yp[:, 1, :], CPY)
                        else:
                            nc.vector.tensor_copy(box["ys"][:, p, :, :], yp)
                    return (230.0, f)

                def c_dma():
                    nc.sync.dma_start(out=y_d[t], in_=box["ys"])

                for p in range(4):
                    cl.append(c_vh0(p))
                    cl.append(c_vh1(p))
                cl.append((0.0, c_dma))
                return cl

            # ---------------- blocks ----------------
            tail_rot = [(ypp, "y"), (accp, "accA"), (accp, "accB"),
                        (accp, "accC")]
            for bk in range(NBLK):
                if bk == 1:
                    # block-1 scores read the filler-produced q projections:
                    # force-drain any q fillers that block 0 didn't absorb
                    while q_fillers:
                        q_fillers.pop(0)[1]()
                otT = wkp.tile([P, 2, QS, P], BF16, tag="otT", name=f"otT{bk}")
                last = bk == NBLK - 1
                for vh in range(2):
                    accs0 = sweep(bk, vh, 0)
                    asb = save_accs(bk, vh, 0, accs0, "asb")
                    accs1 = sweep(bk, vh, 1)
                    bsb = save_accs(bk, vh, 1, accs1, "bsb")
                    combine(bk, vh, asb, bsb, otT, tail=(last and vh == 1))
                if not last:
                    for t in range(QS):
                        proj_fillers.extend(
                            proj_tile_closures(otT, bk * QS + t, (ypp, "y"),
                                               split_copies=False))
                else:
                    inject(1e9)  # drain leftovers (incl. final transposes)
                    for t in range(QS):
                        for est, f in proj_tile_closures(
                                otT, bk * QS + t, tail_rot[t % 4],
                                split_copies=True):
                            f()
    nc.finalize()
    return nc


def _core_inputs(x, w_qkv, w_proj, rms_scale):
    """Host-side shard prep: per-core bf16 weight slices + replicated x^T."""
    bf = mybir.dt.np(BF16)
    ident = np.ascontiguousarray(np.eye(P, dtype=np.float32).astype(bf))
    xt = x.reshape(T, C).T  # [C, T]
    xtr = np.ascontiguousarray(
        xt.reshape(KS, P, 4, NCH).transpose(1, 2, 0, 3).astype(bf)
    )
    sv = np.tile(
        rms_scale.astype(np.float32) * np.float32(1.0 - LAMBDA_INIT), 2
    )  # [256], per-row scale for this core's w_proj rows
    maps = []
    for c in range(N_CORES):
        cols = [
            w_qkv[:, 0 * 1024 + c * P:0 * 1024 + (c + 1) * P],  # q1 heads 2c,2c+1
            w_qkv[:, 1 * 1024 + c * P:1 * 1024 + (c + 1) * P],  # q2
            w_qkv[:, 2 * 1024 + c * P:2 * 1024 + (c + 1) * P],  # k1
            w_qkv[:, 3 * 1024 + c * P:3 * 1024 + (c + 1) * P],  # k2
        ]
        wqk = np.stack(cols, axis=0)  # [4, C, 128]
        wqk = np.ascontiguousarray(
            wqk.reshape(4, KS, P, P).transpose(2, 0, 1, 3).astype(bf)
        )
        wv = w_qkv[:, 2 * C + c * 2 * D2:2 * C + (c + 1) * 2 * D2]  # [C, 256]
        wv = np.ascontiguousarray(
            wv.reshape(KS, P, 2 * D2).transpose(1, 0, 2).astype(bf)
        )
        wp = w_proj[c * 2 * D2:(c + 1) * 2 * D2, :] * sv[:, None]  # [256, T]
        wp = np.ascontiguousarray(
            wp.reshape(2, P, T).transpose(1, 0, 2).astype(bf)
        )
        maps.append({"xt": xtr, "wqk": wqk, "wv": wv, "wp": wp, "ident": ident})
    return maps


def kernel(x, w_qkv, w_proj, lambda_q1, lambda_k1, lambda_q2, lambda_k2, rms_scale):
    from concourse.bass_utils import run_bass_kernel_spmd

    x = np.asarray(x, dtype=np.float32)
    w_qkv = np.asarray(w_qkv, dtype=np.float32)
    w_proj = np.asarray(w_proj, dtype=np.float32)
    rms_scale = np.asarray(rms_scale, dtype=np.float32)
    lam1 = np.exp(np.sum(np.asarray(lambda_q1) * np.asarray(lambda_k1), dtype=np.float32))
    lam2 = np.exp(np.sum(np.asarray(lambda_q2) * np.asarray(lambda_k2), dtype=np.float32))
    lam = float(lam1 - lam2 + LAMBDA_INIT)

    nc = build(lam)
    in_maps = _core_inputs(x, w_qkv, w_proj, rms_scale)
    res = run_bass_kernel_spmd(nc, in_maps, core_ids=list(range(N_CORES)))
    y = np.zeros((TT, P, T), np.float32)
    for rmap in res.results:
        y += np.asarray(rmap["y"], np.float32)
    return y.reshape(1, T, C)


# revision 25
# speedup vs baseline: 1.1064x; 1.0242x over previous
"""Trainium2 Bass kernel for DiffSelfAttention (B=1, T=2048, C=2048, 16 v-heads).

Sharding: tensor-parallel over heads across 8 NeuronCores. Core c owns
v-heads {2c, 2c+1} plus the matching q/k heads of both differential branches.
Each core computes its qkv slice, the attention for its 4 q/k head-pairs, the
differential + per-head RMSNorm, and a partial projection
y_c = out_c @ w_proj[rows_c]. The host sums the 8 partials (unshard step).

v2 design notes (vs the fp32r v1):
  - Everything bf16 on the PE (1 cycle/row at ANY moving size, halved DMA
    and SBUF footprint). Host converts inputs; rel-err budget is 2e-2 and
    bf16 keeps us ~1e-2 or better.
  - Transposed PV: attention is computed as a^T[tq,d2] = et^T @ [v|1] with
    the exp'd scores as the STATIONARY operand and [v | ones-column] as a
    129-wide moving operand. This gets the softmax denominator r in the
    same matmul (column 128) AND puts r on the partition axis, so all the
    differential-combine scalars are per-partition [P,1] operands — no
    broadcast matmuls. v1's separate ones-colsum (131k cycles/core) is gone.
  - Softmax divisions eliminated as in v1: RMSNorm is invariant to any
    per-column positive scale, so o' = a1*r2 - lam*a2*r1 feeds the norm.
  - RMS rsqrt = exp(-0.5*ln(mean)) on ACT, batched [P,8] per (block,head)
    to bound Ln/Exp table swaps (Ln and Exp are in different ACT table
    sets on this compiler: per-element ln/exp costs a 1.3us table load).
    rms_scale * (1-lambda_init) is folded into w_proj rows on the host.
  - o'[tq,d2] is transposed back to [d2,tq] for the projection on the PE
    (identity-matmul transpose). DmaTransposeAnt and tensor_tensor_reduce
    both kill the exec unit on this runtime - do not use them.
  - Work is emitted in 2 tq-blocks of 1024; the q projections for the
    second half, the deferred transposes, and the first block's output
    projection are injected as fillers into the (ACT-bound) attention
    sweeps so the PE never idles. Transposes are injected in PAIRS to
    keep the 2-slot scores-psum rotation aligned, and only from k-iter 8
    so their DVE deps are ready.
  - PSUM: one accumulation group per 2KB bank (start=True lazily zeroes
    the whole bank): scores [P,1024]x2 = 4 banks, pv accumulators
    (8 x [P,132] packed 3-per-bank, one group per bank) = 3, proj = 1.
    The tail projection steals the 3 freed accumulator banks.
  - Input DMAs split across the two HWDGE queues (SP: weights,
    ACT: x chunks) so the transfers overlap; ~12us to first matmul.
"""

import math

import numpy as np

import concourse.bass as bass
import concourse.bacc as bacc
import concourse.mybir as mybir
import concourse.tile as tile

F32 = mybir.dt.float32
BF16 = mybir.dt.bfloat16

T = 2048
C = 2048
N_HEAD = 16
H_DIM = 64
D2 = 2 * H_DIM  # 128 (v-head dim, also the RMS group size)
LAMBDA_INIT = 0.8 - 0.6 * math.exp(-0.3)
SCALE = 1.0 / math.sqrt(H_DIM)
P = 128
KS = C // P  # 16 contraction slabs
TT = T // P  # 16 t-tiles
NCH = 512  # phase-1 t-chunk width
QS = 8  # tq slabs per block
NBLK = 2  # tq blocks of 1024
N_CORES = 8

EXP = mybir.ActivationFunctionType.Exp
LOG = mybir.ActivationFunctionType.Ln
CPY = mybir.ActivationFunctionType.Copy
MULT = mybir.AluOpType.mult
ADD = mybir.AluOpType.add


def build(lam: float) -> bass.Bass:
    nc = bacc.Bacc("TRN2", target_bir_lowering=False, debug=False)

    xb_d = nc.dram_tensor("xt", [P, 4, KS, NCH], BF16, kind="ExternalInput")
    wqk_d = nc.dram_tensor("wqk", [P, 4, KS, P], BF16, kind="ExternalInput")
    wv_d = nc.dram_tensor("wv", [P, KS, 2 * D2], BF16, kind="ExternalInput")
    wp_d = nc.dram_tensor("wp", [P, 2, T], BF16, kind="ExternalInput")
    id_d = nc.dram_tensor("ident", [P, P], BF16, kind="ExternalInput")
    y_d = nc.dram_tensor("y", [TT, P, T], BF16, kind="ExternalOutput")

    with tile.TileContext(nc) as tc:
        with tc.tile_pool(name="persist", bufs=1) as pp, \
             tc.tile_pool(name="etp", bufs=3) as etp, \
             tc.tile_pool(name="work", bufs=2) as wkp, \
             tc.tile_pool(name="ysp", bufs=2) as ysp, \
             tc.tile_pool(name="sc", bufs=2, space="PSUM") as scp, \
             tc.tile_pool(name="acc", bufs=1, space="PSUM") as accp, \
             tc.tile_pool(name="yp", bufs=1, space="PSUM") as ypp:

            xb = pp.tile([P, 4, KS, NCH], BF16)
            wqk = pp.tile([P, 4, KS, P], BF16)
            wv = pp.tile([P, KS, 2 * D2], BF16)
            wp = pp.tile([P, 2, T], BF16)
            qk = pp.tile([P, 4, T], BF16)  # m: q1|q2|k1|k2, [d, T] layout
            ident = pp.tile([P, P], BF16)
            vb = pp.tile([P, KS, 2, 130], BF16)  # [tk, kslab, vh, v|1|pad]

            nc.scalar.dma_start(out=xb[:, 0, 0:8], in_=xb_d[:, 0, 0:8])
            nc.scalar.dma_start(out=xb[:, 0, 8:16], in_=xb_d[:, 0, 8:16])
            nc.scalar.dma_start(out=xb[:, 2], in_=xb_d[:, 2])
            nc.sync.dma_start(out=wqk[:, 2], in_=wqk_d[:, 2])  # k1 first
            nc.sync.dma_start(out=wqk[:, 3], in_=wqk_d[:, 3])
            nc.sync.dma_start(out=xb[:, 1], in_=xb_d[:, 1])
            nc.sync.dma_start(out=wv, in_=wv_d[:])
            nc.sync.dma_start(out=wqk[:, 0], in_=wqk_d[:, 0])
            nc.sync.dma_start(out=wqk[:, 1], in_=wqk_d[:, 1])
            nc.sync.dma_start(out=xb[:, 3], in_=xb_d[:, 3])
            nc.sync.dma_start(out=wp, in_=wp_d[:])
            nc.sync.dma_start(out=ident, in_=id_d[:])
            nc.gpsimd.memset(vb[:, :, :, D2:D2 + 1], 1.0)

            # ---------------- phase 1: qkv projections ----------------
            # (PSUM->SBUF copies ride the ACT engine, idle until the first
            # exp; GPSIMD/Pool cannot read PSUM)
            def emit_qkv_m(n, m):
                ps = scp.tile([P, NCH], F32, tag="s", name=f"psq{n}{m}")
                for k in range(KS):
                    nc.tensor.matmul(
                        ps,
                        wqk[:, m, k, :],
                        xb[:, n, k, :],
                        start=(k == 0),
                        stop=(k == KS - 1),
                    )
                nc.scalar.activation(qk[:, m, n * NCH:(n + 1) * NCH], ps, CPY)

            def emit_v(n, t2):
                g = 4 * n + t2
                ps = scp.tile([P, 2, D2], F32, tag="s", name=f"psv{g}")
                for k in range(KS):
                    nc.tensor.matmul(
                        ps,
                        xb[:, n, k, t2 * P:(t2 + 1) * P],
                        wv[:, k, :],
                        start=(k == 0),
                        stop=(k == KS - 1),
                    )
                nc.scalar.activation(vb[:, g, :, 0:D2], ps, CPY)

            for n in range(4):
                for m in (2, 3):  # k1, k2 (stationaries for all sweeps)
                    emit_qkv_m(n, m)
                for t2 in range(4):
                    emit_v(n, t2)
            for n in (0, 1):
                for m in (0, 1):  # q1, q2 for block 0
                    emit_qkv_m(n, m)

            # ---- filler machinery: PE work injected into ACT-bound sweeps
            q_fillers = []  # q projections for tq-block 1 (drain in block 0)
            tp_fillers = []  # deferred PE transposes (gated: DVE deps lag)
            proj_fillers = []  # block-0 output projection (drain in block 1)

            def inject(budget_ns, tp_ok=True):
                while budget_ns > 0:
                    if q_fillers:
                        lst = q_fillers
                    elif tp_fillers:
                        if not tp_ok:
                            return  # keep ordering: proj waits for tps
                        lst = tp_fillers
                    elif proj_fillers:
                        lst = proj_fillers
                    else:
                        return
                    est, f = lst.pop(0)
                    f()
                    budget_ns -= est

            def queue_q_fillers(n, m):
                # q projections for tq-block 1, chunk n, using the yp psum
                # slot (idle during block-0 sweeps)
                box = {}

                def mk(k):
                    def f():
                        if k == 0:
                            box["ps"] = ypp.tile(
                                [P, NCH], F32, tag="y", name=f"psq{n}{m}"
                            )
                        nc.tensor.matmul(
                            box["ps"],
                            wqk[:, m, k, :],
                            xb[:, n, k, :],
                            start=(k == 0),
                            stop=(k == KS - 1),
                        )
                        if k == KS - 1:
                            nc.vector.tensor_copy(
                                qk[:, m, n * NCH:(n + 1) * NCH], box["ps"]
                            )

                    return (220.0, f)

                q_fillers.extend(mk(k) for k in range(KS))

            for n in (2, 3):
                for m in (0, 1):
                    queue_q_fillers(n, m)

            def emit_tp(o2, vh, q, otT, psum_src=None, act_copy=False):
                pool, tag = psum_src if psum_src else (scp, "s")
                pt = pool.tile([P, P], BF16, tag=tag, name=f"tp{vh}{q}")
                nc.tensor.transpose(pt, o2[:, q, :], ident)
                if act_copy:
                    nc.scalar.activation(otT[:, vh, q, :], pt, CPY)
                else:
                    nc.vector.tensor_copy(otT[:, vh, q, :], pt)

            def queue_tp_pairs(o2, vh, otT):
                for q0 in range(0, QS, 2):
                    def f(q0=q0):
                        emit_tp(o2, vh, q0, otT)
                        emit_tp(o2, vh, q0 + 1, otT)
                    tp_fillers.append((120.0, f))

            # ---------------- phase 2: attention sweeps ----------------
            def get_accs(bk, vh, br):
                a = accp.tile([P, 3, 132], F32, tag="accA", name=f"accA{bk}{vh}{br}")
                b = accp.tile([P, 3, 132], F32, tag="accB", name=f"accB{bk}{vh}{br}")
                c = accp.tile([P, 2, 132], F32, tag="accC", name=f"accC{bk}{vh}{br}")
                return [(a, 0), (a, 1), (a, 2), (b, 0), (b, 1), (b, 2), (c, 0), (c, 1)]

            def sweep(bk, vh, br):
                # scores + exp + pv for head-pair (vh,br), tq block bk,
                # software-pipelined one k-slab ahead so the PE never waits
                # a full exp latency
                rows = slice(vh * H_DIM, (vh + 1) * H_DIM)
                accs = get_accs(bk, vh, br)
                ets = [None] * TT
                for j in range(TT + 1):
                    if j < TT:
                        ps = scp.tile(
                            [P, 2 * NCH], F32, tag="s", name=f"sc{bk}{vh}{br}{j}"
                        )
                        for i in range(2):
                            nc.tensor.matmul(
                                ps[:, i * NCH:(i + 1) * NCH],
                                qk[rows, 2 + br, j * P:(j + 1) * P],
                                qk[rows, br, bk * 1024 + i * NCH:
                                   bk * 1024 + (i + 1) * NCH],
                                start=True,
                                stop=True,
                            )
                        et = etp.tile(
                            [P, 2 * NCH], BF16, tag="e", name=f"et{bk}{vh}{br}{j}"
                        )
                        nc.scalar.activation(et, ps, EXP, scale=SCALE)
                        ets[j] = et
                    if j > 0:
                        inject(900.0 if j == 1 else 220.0, tp_ok=(j >= 8))
                        et = ets[j - 1]
                        for q in range(QS):
                            at, qi = accs[q]
                            # one psum accumulation group per BANK: start
                            # zeroes the whole 2KB zero-region lazily
                            first = (j - 1 == 0) and qi == 0
                            lastq = qi == (3 if q < 6 else 2) - 1
                            nc.tensor.matmul(
                                at[:, qi, 0:129],
                                et[:, q * P:(q + 1) * P],
                                vb[:, j - 1, vh, 0:129],
                                start=first,
                                stop=(j - 1 == TT - 1) and lastq,
                            )
                        ets[j - 1] = None
                return accs

            def save_accs(bk, vh, br, accs, tag):
                # copy a|r psum accumulators to SBUF (frees the acc banks
                # for the next sweep after ~3 DVE ops)
                sb = wkp.tile([P, QS, 132], F32, tag=tag, name=f"{tag}{bk}{vh}")
                nc.vector.tensor_copy(sb[:, 0:3, 0:129], accs[0][0][:, :, 0:129])
                nc.scalar.activation(sb[:, 3:6, 0:129], accs[3][0][:, :, 0:129],
                                     CPY)
                nc.vector.tensor_copy(sb[:, 6:8, 0:129], accs[6][0][:, :, 0:129])
                return sb

            SQR = mybir.ActivationFunctionType.Square

            def combine(bk, vh, asb, bsb, otT, tail=False):
                # o' = a1*r2 - lam*a2*r1 (per-column rescale of the true o;
                # RMSNorm cancels it), then per-head RMS + bf16 + transpose.
                r1n = wkp.tile([P, QS, 1], F32, tag="r1n", name=f"r1n{bk}{vh}")
                o12 = wkp.tile([P, QS, P], F32, tag="o12", name=f"o12{bk}{vh}")
                sqs = wkp.tile([P, P], F32, tag="sqs", name=f"sqs{bk}{vh}")
                msb = wkp.tile([P, QS, 1], F32, tag="msb", name=f"msb{bk}{vh}")
                lns = wkp.tile([P, QS, 1], F32, tag="lns", name=f"lns{bk}{vh}")
                rs = wkp.tile([P, QS, 1], F32, tag="rs", name=f"rs{bk}{vh}")
                o2 = wkp.tile([P, QS, P], BF16, tag="o2", name=f"o2{bk}{vh}")
                nc.vector.tensor_scalar_mul(r1n, asb[:, :, 128:129], -lam)

                I32 = mybir.dt.int32
                SHR = mybir.AluOpType.logical_shift_right

                def rms_tail(qs):
                    # rs = rsqrt(msb) via bit-hack seed + 2 Newton steps
                    nc.vector.tensor_scalar(
                        out=lns[:, qs, :].bitcast(I32),
                        in0=msb[:, qs, :].bitcast(I32),
                        scalar1=1, scalar2=None, op0=SHR)
                    nc.vector.tensor_scalar(
                        out=rs[:, qs, :].bitcast(I32),
                        in0=lns[:, qs, :].bitcast(I32),
                        scalar1=-1, scalar2=0x5F3759DF, op0=MULT, op1=ADD)
                    for _ in range(2):
                        nc.vector.tensor_mul(lns[:, qs, :], rs[:, qs, :],
                                             rs[:, qs, :])
                        nc.vector.tensor_mul(lns[:, qs, :], lns[:, qs, :],
                                             msb[:, qs, :])
                        nc.vector.tensor_scalar(
                            out=lns[:, qs, :], in0=lns[:, qs, :],
                            scalar1=-0.5, scalar2=1.5, op0=MULT, op1=ADD)
                        nc.vector.tensor_mul(rs[:, qs, :], rs[:, qs, :],
                                             lns[:, qs, :])

                for q in range(QS):
                    nc.vector.tensor_scalar_mul(
                        o12[:, q, :], asb[:, q, 0:P], bsb[:, q, 128:129]
                    )
                    nc.vector.scalar_tensor_tensor(
                        o12[:, q, :], bsb[:, q, 0:P], r1n[:, q, :], o12[:, q, :],
                        op0=MULT, op1=ADD,
                    )
                    if tail:
                        nc.scalar.activation(sqs, o12[:, q, :], SQR,
                                             accum_out=msb[:, q, :])
                    else:
                        nc.vector.tensor_mul(sqs, o12[:, q, :], o12[:, q, :])
                        nc.vector.tensor_reduce(
                            msb[:, q, :], sqs, mybir.AxisListType.X, ADD
                        )
                rms_tail(slice(0, QS))
                for q in range(QS):
                    if tail:
                        nc.scalar.activation(o2[:, q, :], o12[:, q, :], CPY,
                                             scale=rs[:, q, :])
                    else:
                        nc.vector.tensor_scalar_mul(
                            o2[:, q, :], o12[:, q, :], rs[:, q, :]
                        )
                queue_tp_pairs(o2, vh, otT)

            # -------- output projection for one 128-row tq tile ----------
            # psum_src: (pool, tag) for this tile's psum bank. Mid-kernel
            # (filler path) only the 1-bank "y" tag is free and copies go to
            # DVE; at the tail the attention accumulator banks are stolen
            # for a 4-bank rotation and copies split DVE/ACT.
            def proj_tile_closures(otT, t, psum_src, split_copies):
                q = t % QS
                pool, tag = psum_src
                box = {}
                cl = []

                def c_vh0(p):
                    def f():
                        if p == 0:
                            box["yp"] = pool.tile([P, 2, 256], F32, tag=tag,
                                                  name=f"yt{t}")
                            box["ys"] = ysp.tile([P, 4, 2, 256], BF16,
                                                 tag="ysb", name=f"ys{t}")
                        yp = box["yp"]
                        for r in range(2):
                            nc.tensor.matmul(
                                yp[:, r, :],
                                otT[:, 0, q, :],
                                wp[:, 0, 512 * p + 256 * r:512 * p + 256 * (r + 1)],
                                start=(r == 0),
                                stop=False,
                            )
                    return (230.0, f)

                def c_vh1(p):
                    def f():
                        yp = box["yp"]
                        for r in range(2):
                            nc.tensor.matmul(
                                yp[:, r, :],
                                otT[:, 1, q, :],
                                wp[:, 1, 512 * p + 256 * r:512 * p + 256 * (r + 1)],
                                start=False,
                                stop=(r == 1),
                            )
                        if split_copies:
                            nc.vector.tensor_copy(box["ys"][:, p, 0, :],
                                                  yp[:, 0, :])
                            nc.scalar.activation(box["ys"][:, p, 1, :],
                                                 yp[:, 1, :], CPY)
                        else:
                            nc.vector.tensor_copy(box["ys"][:, p, :, :], yp)
                    return (230.0, f)

                def c_dma():
                    nc.sync.dma_start(out=y_d[t], in_=box["ys"])

                for p in range(4):
                    cl.append(c_vh0(p))
                    cl.append(c_vh1(p))
                cl.append((0.0, c_dma))
                return cl

            # ---------------- blocks ----------------
            tail_rot = [(ypp, "y"), (accp, "accA"), (accp, "accB"),
                        (accp, "accC")]
            for bk in range(NBLK):
                if bk == 1:
                    # block-1 scores read the filler-produced q projections:
                    # force-drain any q fillers that block 0 didn't absorb
                    while q_fillers:
                        q_fillers.pop(0)[1]()
                otT = wkp.tile([P, 2, QS, P], BF16, tag="otT", name=f"otT{bk}")
                last = bk == NBLK - 1
                for vh in range(2):
                    accs0 = sweep(bk, vh, 0)
                    asb = save_accs(bk, vh, 0, accs0, "asb")
                    accs1 = sweep(bk, vh, 1)
                    bsb = save_accs(bk, vh, 1, accs1, "bsb")
                    combine(bk, vh, asb, bsb, otT, tail=(last and vh == 1))
                if not last:
                    for t in range(QS):
                        proj_fillers.extend(
                            proj_tile_closures(otT, bk * QS + t, (ypp, "y"),
                                               split_copies=False))
                else:
                    inject(1e9)  # drain leftovers (incl. final transposes)
                    for t in range(QS):
                        for est, f in proj_tile_closures(
                                otT, bk * QS + t, tail_rot[t % 4],
                                split_copies=True):
                            f()
    nc.finalize()
    return nc


def _core_inputs(x, w_qkv, w_proj, rms_scale):
    """Host-side shard prep: per-core bf16 weight slices + replicated x^T."""
    bf = mybir.dt.np(BF16)
    ident = np.ascontiguousarray(np.eye(P, dtype=np.float32).astype(bf))
    xt = x.reshape(T, C).T  # [C, T]
    xtr = np.ascontiguousarray(
        xt.reshape(KS, P, 4, NCH).transpose(1, 2, 0, 3).astype(bf)
    )
    sv = np.tile(
        rms_scale.astype(np.float32) * np.float32(1.0 - LAMBDA_INIT)
        * np.float32(math.sqrt(D2)), 2
    )  # [256]; sqrt(D2) because the kernel's rsqrt takes the SUM of squares
    maps = []
    for c in range(N_CORES):
        cols = [
            w_qkv[:, 0 * 1024 + c * P:0 * 1024 + (c + 1) * P],  # q1 heads 2c,2c+1
            w_qkv[:, 1 * 1024 + c * P:1 * 1024 + (c + 1) * P],  # q2
            w_qkv[:, 2 * 1024 + c * P:2 * 1024 + (c + 1) * P],  # k1
            w_qkv[:, 3 * 1024 + c * P:3 * 1024 + (c + 1) * P],  # k2
        ]
        wqk = np.stack(cols, axis=0)  # [4, C, 128]
        wqk = np.ascontiguousarray(
            wqk.reshape(4, KS, P, P).transpose(2, 0, 1, 3).astype(bf)
        )
        wv = w_qkv[:, 2 * C + c * 2 * D2:2 * C + (c + 1) * 2 * D2]  # [C, 256]
        wv = np.ascontiguousarray(
            wv.reshape(KS, P, 2 * D2).transpose(1, 0, 2).astype(bf)
        )
        wp = w_proj[c * 2 * D2:(c + 1) * 2 * D2, :] * sv[:, None]  # [256, T]
        wp = np.ascontiguousarray(
            wp.reshape(2, P, T).transpose(1, 0, 2).astype(bf)
        )
        maps.append({"xt": xtr, "wqk": wqk, "wv": wv, "wp": wp, "ident": ident})
    return maps


def kernel(x, w_qkv, w_proj, lambda_q1, lambda_k1, lambda_q2, lambda_k2, rms_scale):
    from concourse.bass_utils import run_bass_kernel_spmd

    x = np.asarray(x, dtype=np.float32)
    w_qkv = np.asarray(w_qkv, dtype=np.float32)
    w_proj = np.asarray(w_proj, dtype=np.float32)
    rms_scale = np.asarray(rms_scale, dtype=np.float32)
    lam1 = np.exp(np.sum(np.asarray(lambda_q1) * np.asarray(lambda_k1), dtype=np.float32))
    lam2 = np.exp(np.sum(np.asarray(lambda_q2) * np.asarray(lambda_k2), dtype=np.float32))
    lam = float(lam1 - lam2 + LAMBDA_INIT)

    nc = build(lam)
    in_maps = _core_inputs(x, w_qkv, w_proj, rms_scale)
    res = run_bass_kernel_spmd(nc, in_maps, core_ids=list(range(N_CORES)))
    y = np.zeros((TT, P, T), np.float32)
    for rmap in res.results:
        y += np.asarray(rmap["y"], np.float32)
    return y.reshape(1, T, C)


# revision 26
# speedup vs baseline: 1.1121x; 1.0052x over previous
"""Trainium2 Bass kernel for DiffSelfAttention (B=1, T=2048, C=2048, 16 v-heads).

Sharding: tensor-parallel over heads across 8 NeuronCores. Core c owns
v-heads {2c, 2c+1} plus the matching q/k heads of both differential branches.
Each core computes its qkv slice, the attention for its 4 q/k head-pairs, the
differential + per-head RMSNorm, and a partial projection
y_c = out_c @ w_proj[rows_c]. The host sums the 8 partials (unshard step).

v2 design notes (vs the fp32r v1):
  - Everything bf16 on the PE (1 cycle/row at ANY moving size, halved DMA
    and SBUF footprint). Host converts inputs; rel-err budget is 2e-2 and
    bf16 keeps us ~1e-2 or better.
  - Transposed PV: attention is computed as a^T[tq,d2] = et^T @ [v|1] with
    the exp'd scores as the STATIONARY operand and [v | ones-column] as a
    129-wide moving operand. This gets the softmax denominator r in the
    same matmul (column 128) AND puts r on the partition axis, so all the
    differential-combine scalars are per-partition [P,1] operands — no
    broadcast matmuls. v1's separate ones-colsum (131k cycles/core) is gone.
  - Softmax divisions eliminated as in v1: RMSNorm is invariant to any
    per-column positive scale, so o' = a1*r2 - lam*a2*r1 feeds the norm.
  - RMS rsqrt = exp(-0.5*ln(mean)) on ACT, batched [P,8] per (block,head)
    to bound Ln/Exp table swaps (Ln and Exp are in different ACT table
    sets on this compiler: per-element ln/exp costs a 1.3us table load).
    rms_scale * (1-lambda_init) is folded into w_proj rows on the host.
  - o'[tq,d2] is transposed back to [d2,tq] for the projection on the PE
    (identity-matmul transpose). DmaTransposeAnt and tensor_tensor_reduce
    both kill the exec unit on this runtime - do not use them.
  - Work is emitted in 2 tq-blocks of 1024; the q projections for the
    second half, the deferred transposes, and the first block's output
    projection are injected as fillers into the (ACT-bound) attention
    sweeps so the PE never idles. Transposes are injected in PAIRS to
    keep the 2-slot scores-psum rotation aligned, and only from k-iter 8
    so their DVE deps are ready.
  - PSUM: one accumulation group per 2KB bank (start=True lazily zeroes
    the whole bank): scores [P,1024]x2 = 4 banks, pv accumulators
    (8 x [P,132] packed 3-per-bank, one group per bank) = 3, proj = 1.
    The tail projection steals the 3 freed accumulator banks.
  - Input DMAs split across the two HWDGE queues (SP: weights,
    ACT: x chunks) so the transfers overlap; ~12us to first matmul.
"""

import math

import numpy as np

import concourse.bass as bass
import concourse.bacc as bacc
import concourse.mybir as mybir
import concourse.tile as tile

F32 = mybir.dt.float32
BF16 = mybir.dt.bfloat16

T = 2048
C = 2048
N_HEAD = 16
H_DIM = 64
D2 = 2 * H_DIM  # 128 (v-head dim, also the RMS group size)
LAMBDA_INIT = 0.8 - 0.6 * math.exp(-0.3)
SCALE = 1.0 / math.sqrt(H_DIM)
P = 128
KS = C // P  # 16 contraction slabs
TT = T // P  # 16 t-tiles
NCH = 512  # phase-1 t-chunk width
QS = 8  # tq slabs per block
NBLK = 2  # tq blocks of 1024
N_CORES = 8

EXP = mybir.ActivationFunctionType.Exp
LOG = mybir.ActivationFunctionType.Ln
CPY = mybir.ActivationFunctionType.Copy
MULT = mybir.AluOpType.mult
ADD = mybir.AluOpType.add


def build(lam: float) -> bass.Bass:
    nc = bacc.Bacc("TRN2", target_bir_lowering=False, debug=False)

    xb_d = nc.dram_tensor("xt", [P, 4, KS, NCH], BF16, kind="ExternalInput")
    wqk_d = nc.dram_tensor("wqk", [P, 4, KS, P], BF16, kind="ExternalInput")
    wv_d = nc.dram_tensor("wv", [P, KS, 2 * D2], BF16, kind="ExternalInput")
    wp_d = nc.dram_tensor("wp", [P, 2, T], BF16, kind="ExternalInput")
    id_d = nc.dram_tensor("ident", [P, P], BF16, kind="ExternalInput")
    y_d = nc.dram_tensor("y", [TT, P, T], BF16, kind="ExternalOutput")

    with tile.TileContext(nc) as tc:
        with tc.tile_pool(name="persist", bufs=1) as pp, \
             tc.tile_pool(name="etp", bufs=3) as etp, \
             tc.tile_pool(name="work", bufs=2) as wkp, \
             tc.tile_pool(name="ysp", bufs=2) as ysp, \
             tc.tile_pool(name="sc", bufs=2, space="PSUM") as scp, \
             tc.tile_pool(name="acc", bufs=1, space="PSUM") as accp, \
             tc.tile_pool(name="yp", bufs=1, space="PSUM") as ypp:

            xb = pp.tile([P, 4, KS, NCH], BF16)
            wqk = pp.tile([P, 4, KS, P], BF16)
            wv = pp.tile([P, KS, 2 * D2], BF16)
            wp = pp.tile([P, 2, T], BF16)
            qk = pp.tile([P, 4, T], BF16)  # m: q1|q2|k1|k2, [d, T] layout
            ident = pp.tile([P, P], BF16)
            vb = pp.tile([P, KS, 2, 130], BF16)  # [tk, kslab, vh, v|1|pad]

            # chunks in consumption order on the ACT queue; weights (then
            # later chunks) on the SP queue — the queues share HBM bandwidth,
            # so loading a late chunk early starves an earlier one
            nc.scalar.dma_start(out=xb[:, 0, 0:8], in_=xb_d[:, 0, 0:8])
            nc.scalar.dma_start(out=xb[:, 0, 8:16], in_=xb_d[:, 0, 8:16])
            nc.scalar.dma_start(out=xb[:, 1], in_=xb_d[:, 1])
            nc.sync.dma_start(out=wqk[:, 2], in_=wqk_d[:, 2])  # k1 first
            nc.sync.dma_start(out=wqk[:, 3], in_=wqk_d[:, 3])
            nc.sync.dma_start(out=wv, in_=wv_d[:])
            nc.sync.dma_start(out=wqk[:, 0], in_=wqk_d[:, 0])
            nc.sync.dma_start(out=wqk[:, 1], in_=wqk_d[:, 1])
            nc.sync.dma_start(out=xb[:, 2], in_=xb_d[:, 2])
            nc.sync.dma_start(out=xb[:, 3], in_=xb_d[:, 3])
            nc.sync.dma_start(out=wp, in_=wp_d[:])
            nc.sync.dma_start(out=ident, in_=id_d[:])
            nc.gpsimd.memset(vb[:, :, :, D2:D2 + 1], 1.0)

            # ---------------- phase 1: qkv projections ----------------
            # (PSUM->SBUF copies ride the ACT engine, idle until the first
            # exp; GPSIMD/Pool cannot read PSUM)
            def emit_qkv_m(n, m):
                ps = scp.tile([P, NCH], F32, tag="s", name=f"psq{n}{m}")
                for k in range(KS):
                    nc.tensor.matmul(
                        ps,
                        wqk[:, m, k, :],
                        xb[:, n, k, :],
                        start=(k == 0),
                        stop=(k == KS - 1),
                    )
                nc.scalar.activation(qk[:, m, n * NCH:(n + 1) * NCH], ps, CPY)

            def emit_v(n, t2):
                g = 4 * n + t2
                ps = scp.tile([P, 2, D2], F32, tag="s", name=f"psv{g}")
                for k in range(KS):
                    nc.tensor.matmul(
                        ps,
                        xb[:, n, k, t2 * P:(t2 + 1) * P],
                        wv[:, k, :],
                        start=(k == 0),
                        stop=(k == KS - 1),
                    )
                nc.scalar.activation(vb[:, g, :, 0:D2], ps, CPY)

            for n in range(4):
                for m in (2, 3):  # k1, k2 (stationaries for all sweeps)
                    emit_qkv_m(n, m)
                for t2 in range(4):
                    emit_v(n, t2)
            for n in (0, 1):
                for m in (0, 1):  # q1, q2 for block 0
                    emit_qkv_m(n, m)

            # ---- filler machinery: PE work injected into ACT-bound sweeps
            q_fillers = []  # q projections for tq-block 1 (drain in block 0)
            tp_fillers = []  # deferred PE transposes (gated: DVE deps lag)
            proj_fillers = []  # block-0 output projection (drain in block 1)

            def inject(budget_ns, tp_ok=True):
                while budget_ns > 0:
                    if q_fillers:
                        lst = q_fillers
                    elif tp_fillers:
                        if not tp_ok:
                            return  # keep ordering: proj waits for tps
                        lst = tp_fillers
                    elif proj_fillers:
                        lst = proj_fillers
                    else:
                        return
                    est, f = lst.pop(0)
                    f()
                    budget_ns -= est

            def queue_q_fillers(n, m):
                # q projections for tq-block 1, chunk n, using the yp psum
                # slot (idle during block-0 sweeps)
                box = {}

                def mk(k):
                    def f():
                        if k == 0:
                            box["ps"] = ypp.tile(
                                [P, NCH], F32, tag="y", name=f"psq{n}{m}"
                            )
                        nc.tensor.matmul(
                            box["ps"],
                            wqk[:, m, k, :],
                            xb[:, n, k, :],
                            start=(k == 0),
                            stop=(k == KS - 1),
                        )
                        if k == KS - 1:
                            nc.vector.tensor_copy(
                                qk[:, m, n * NCH:(n + 1) * NCH], box["ps"]
                            )

                    return (220.0, f)

                q_fillers.extend(mk(k) for k in range(KS))

            for n in (2, 3):
                for m in (0, 1):
                    queue_q_fillers(n, m)

            def emit_tp(o2, vh, q, otT, psum_src=None, act_copy=False):
                pool, tag = psum_src if psum_src else (scp, "s")
                pt = pool.tile([P, P], BF16, tag=tag, name=f"tp{vh}{q}")
                nc.tensor.transpose(pt, o2[:, q, :], ident)
                if act_copy:
                    nc.scalar.activation(otT[:, vh, q, :], pt, CPY)
                else:
                    nc.vector.tensor_copy(otT[:, vh, q, :], pt)

            def queue_tp_pairs(o2, vh, otT):
                for q0 in range(0, QS, 2):
                    def f(q0=q0):
                        emit_tp(o2, vh, q0, otT)
                        emit_tp(o2, vh, q0 + 1, otT)
                    tp_fillers.append((120.0, f))

            # ---------------- phase 2: attention sweeps ----------------
            def get_accs(bk, vh, br):
                a = accp.tile([P, 3, 132], F32, tag="accA", name=f"accA{bk}{vh}{br}")
                b = accp.tile([P, 3, 132], F32, tag="accB", name=f"accB{bk}{vh}{br}")
                c = accp.tile([P, 2, 132], F32, tag="accC", name=f"accC{bk}{vh}{br}")
                return [(a, 0), (a, 1), (a, 2), (b, 0), (b, 1), (b, 2), (c, 0), (c, 1)]

            def sweep(bk, vh, br):
                # scores + exp + pv for head-pair (vh,br), tq block bk,
                # software-pipelined one k-slab ahead so the PE never waits
                # a full exp latency
                rows = slice(vh * H_DIM, (vh + 1) * H_DIM)
                accs = get_accs(bk, vh, br)
                ets = [None] * TT
                for j in range(TT + 1):
                    if j < TT:
                        ps = scp.tile(
                            [P, 2 * NCH], F32, tag="s", name=f"sc{bk}{vh}{br}{j}"
                        )
                        for i in range(2):
                            nc.tensor.matmul(
                                ps[:, i * NCH:(i + 1) * NCH],
                                qk[rows, 2 + br, j * P:(j + 1) * P],
                                qk[rows, br, bk * 1024 + i * NCH:
                                   bk * 1024 + (i + 1) * NCH],
                                start=True,
                                stop=True,
                            )
                        et = etp.tile(
                            [P, 2 * NCH], BF16, tag="e", name=f"et{bk}{vh}{br}{j}"
                        )
                        nc.scalar.activation(et, ps, EXP, scale=SCALE)
                        ets[j] = et
                    if j > 0:
                        inject(900.0 if j == 1 else 220.0, tp_ok=(j >= 8))
                        et = ets[j - 1]
                        for q in range(QS):
                            at, qi = accs[q]
                            # one psum accumulation group per BANK: start
                            # zeroes the whole 2KB zero-region lazily
                            first = (j - 1 == 0) and qi == 0
                            lastq = qi == (3 if q < 6 else 2) - 1
                            nc.tensor.matmul(
                                at[:, qi, 0:129],
                                et[:, q * P:(q + 1) * P],
                                vb[:, j - 1, vh, 0:129],
                                start=first,
                                stop=(j - 1 == TT - 1) and lastq,
                            )
                        ets[j - 1] = None
                return accs

            def save_accs(bk, vh, br, accs, tag):
                # copy a|r psum accumulators to SBUF (frees the acc banks
                # for the next sweep after ~3 DVE ops)
                sb = wkp.tile([P, QS, 132], F32, tag=tag, name=f"{tag}{bk}{vh}")
                nc.vector.tensor_copy(sb[:, 0:3, 0:129], accs[0][0][:, :, 0:129])
                nc.scalar.activation(sb[:, 3:6, 0:129], accs[3][0][:, :, 0:129],
                                     CPY)
                nc.vector.tensor_copy(sb[:, 6:8, 0:129], accs[6][0][:, :, 0:129])
                return sb

            SQR = mybir.ActivationFunctionType.Square

            def combine(bk, vh, asb, bsb, otT, tail=False):
                # o' = a1*r2 - lam*a2*r1 (per-column rescale of the true o;
                # RMSNorm cancels it), then per-head RMS + bf16 + transpose.
                r1n = wkp.tile([P, QS, 1], F32, tag="r1n", name=f"r1n{bk}{vh}")
                o12 = wkp.tile([P, QS, P], F32, tag="o12", name=f"o12{bk}{vh}")
                sqs = wkp.tile([P, P], F32, tag="sqs", name=f"sqs{bk}{vh}")
                msb = wkp.tile([P, QS, 1], F32, tag="msb", name=f"msb{bk}{vh}")
                lns = wkp.tile([P, QS, 1], F32, tag="lns", name=f"lns{bk}{vh}")
                rs = wkp.tile([P, QS, 1], F32, tag="rs", name=f"rs{bk}{vh}")
                o2 = wkp.tile([P, QS, P], BF16, tag="o2", name=f"o2{bk}{vh}")
                nc.vector.tensor_scalar_mul(r1n, asb[:, :, 128:129], -lam)

                I32 = mybir.dt.int32
                SHR = mybir.AluOpType.logical_shift_right

                def rms_tail(qs):
                    # rs = rsqrt(msb) via bit-hack seed + 2 Newton steps
                    nc.vector.tensor_scalar(
                        out=lns[:, qs, :].bitcast(I32),
                        in0=msb[:, qs, :].bitcast(I32),
                        scalar1=1, scalar2=None, op0=SHR)
                    nc.vector.tensor_scalar(
                        out=rs[:, qs, :].bitcast(I32),
                        in0=lns[:, qs, :].bitcast(I32),
                        scalar1=-1, scalar2=0x5F3759DF, op0=MULT, op1=ADD)
                    for _ in range(2):
                        nc.vector.tensor_mul(lns[:, qs, :], rs[:, qs, :],
                                             rs[:, qs, :])
                        nc.vector.tensor_mul(lns[:, qs, :], lns[:, qs, :],
                                             msb[:, qs, :])
                        nc.vector.tensor_scalar(
                            out=lns[:, qs, :], in0=lns[:, qs, :],
                            scalar1=-0.5, scalar2=1.5, op0=MULT, op1=ADD)
                        nc.vector.tensor_mul(rs[:, qs, :], rs[:, qs, :],
                                             lns[:, qs, :])

                for q in range(QS):
                    nc.vector.tensor_scalar_mul(
                        o12[:, q, :], asb[:, q, 0:P], bsb[:, q, 128:129]
                    )
                    nc.vector.scalar_tensor_tensor(
                        o12[:, q, :], bsb[:, q, 0:P], r1n[:, q, :], o12[:, q, :],
                        op0=MULT, op1=ADD,
                    )
                    if tail:
                        nc.scalar.activation(sqs, o12[:, q, :], SQR,
                                             accum_out=msb[:, q, :])
                    else:
                        nc.vector.tensor_mul(sqs, o12[:, q, :], o12[:, q, :])
                        nc.vector.tensor_reduce(
                            msb[:, q, :], sqs, mybir.AxisListType.X, ADD
                        )
                rms_tail(slice(0, QS))
                for q in range(QS):
                    if tail:
                        nc.scalar.activation(o2[:, q, :], o12[:, q, :], CPY,
                                             scale=rs[:, q, :])
                    else:
                        nc.vector.tensor_scalar_mul(
                            o2[:, q, :], o12[:, q, :], rs[:, q, :]
                        )
                queue_tp_pairs(o2, vh, otT)

            # -------- output projection for one 128-row tq tile ----------
            # psum_src: (pool, tag) for this tile's psum bank. Mid-kernel
            # (filler path) only the 1-bank "y" tag is free and copies go to
            # DVE; at the tail the attention accumulator banks are stolen
            # for a 4-bank rotation and copies split DVE/ACT.
            def proj_tile_closures(otT, t, psum_src, split_copies):
                q = t % QS
                pool, tag = psum_src
                box = {}
                cl = []

                def c_vh0(p):
                    def f():
                        if p == 0:
                            box["yp"] = pool.tile([P, 2, 256], F32, tag=tag,
                                                  name=f"yt{t}")
                            box["ys"] = ysp.tile([P, 4, 2, 256], BF16,
                                                 tag="ysb", name=f"ys{t}")
                        yp = box["yp"]
                        for r in range(2):
                            nc.tensor.matmul(
                                yp[:, r, :],
                                otT[:, 0, q, :],
                                wp[:, 0, 512 * p + 256 * r:512 * p + 256 * (r + 1)],
                                start=(r == 0),
                                stop=False,
                            )
                    return (230.0, f)

                def c_vh1(p):
                    def f():
                        yp = box["yp"]
                        for r in range(2):
                            nc.tensor.matmul(
                                yp[:, r, :],
                                otT[:, 1, q, :],
                                wp[:, 1, 512 * p + 256 * r:512 * p + 256 * (r + 1)],
                                start=False,
                                stop=(r == 1),
                            )
                        if split_copies:
                            nc.vector.tensor_copy(box["ys"][:, p, 0, :],
                                                  yp[:, 0, :])
                            nc.scalar.activation(box["ys"][:, p, 1, :],
                                                 yp[:, 1, :], CPY)
                        else:
                            nc.vector.tensor_copy(box["ys"][:, p, :, :], yp)
                    return (230.0, f)

                def c_dma():
                    nc.sync.dma_start(out=y_d[t], in_=box["ys"])

                for p in range(4):
                    cl.append(c_vh0(p))
                    cl.append(c_vh1(p))
                cl.append((0.0, c_dma))
                return cl

            # ---------------- blocks ----------------
            tail_rot = [(ypp, "y"), (accp, "accA"), (accp, "accB"),
                        (accp, "accC")]
            for bk in range(NBLK):
                if bk == 1:
                    # block-1 scores read the filler-produced q projections:
                    # force-drain any q fillers that block 0 didn't absorb
                    while q_fillers:
                        q_fillers.pop(0)[1]()
                otT = wkp.tile([P, 2, QS, P], BF16, tag="otT", name=f"otT{bk}")
                last = bk == NBLK - 1
                for vh in range(2):
                    accs0 = sweep(bk, vh, 0)
                    asb = save_accs(bk, vh, 0, accs0, "asb")
                    accs1 = sweep(bk, vh, 1)
                    bsb = save_accs(bk, vh, 1, accs1, "bsb")
                    combine(bk, vh, asb, bsb, otT, tail=(last and vh == 1))
                if not last:
                    for t in range(QS):
                        proj_fillers.extend(
                            proj_tile_closures(otT, bk * QS + t, (ypp, "y"),
                                               split_copies=False))
                else:
                    inject(1e9)  # drain leftovers (incl. final transposes)
                    for t in range(QS):
                        for est, f in proj_tile_closures(
                                otT, bk * QS + t, tail_rot[t % 4],
                                split_copies=True):
                            f()
    nc.finalize()
    return nc


def _core_inputs(x, w_qkv, w_proj, rms_scale):
    """Host-side shard prep: per-core bf16 weight slices + replicated x^T."""
    bf = mybir.dt.np(BF16)
    ident = np.ascontiguousarray(np.eye(P, dtype=np.float32).astype(bf))
    xt = x.reshape(T, C).T  # [C, T]
    xtr = np.ascontiguousarray(
        xt.reshape(KS, P, 4, NCH).transpose(1, 2, 0, 3).astype(bf)
    )
    sv = np.tile(
        rms_scale.astype(np.float32) * np.float32(1.0 - LAMBDA_INIT)
        * np.float32(math.sqrt(D2)), 2
    )  # [256]; sqrt(D2) because the kernel's rsqrt takes the SUM of squares
    maps = []
    for c in range(N_CORES):
        cols = [
            w_qkv[:, 0 * 1024 + c * P:0 * 1024 + (c + 1) * P],  # q1 heads 2c,2c+1
            w_qkv[:, 1 * 1024 + c * P:1 * 1024 + (c + 1) * P],  # q2
            w_qkv[:, 2 * 1024 + c * P:2 * 1024 + (c + 1) * P],  # k1
            w_qkv[:, 3 * 1024 + c * P:3 * 1024 + (c + 1) * P],  # k2
        ]
        wqk = np.stack(cols, axis=0)  # [4, C, 128]
        wqk = np.ascontiguousarray(
            wqk.reshape(4, KS, P, P).transpose(2, 0, 1, 3).astype(bf)
        )
        wv = w_qkv[:, 2 * C + c * 2 * D2:2 * C + (c + 1) * 2 * D2]  # [C, 256]
        wv = np.ascontiguousarray(
            wv.reshape(KS, P, 2 * D2).transpose(1, 0, 2).astype(bf)
        )
        wp = w_proj[c * 2 * D2:(c + 1) * 2 * D2, :] * sv[:, None]  # [256, T]
        wp = np.ascontiguousarray(
            wp.reshape(2, P, T).transpose(1, 0, 2).astype(bf)
        )
        maps.append({"xt": xtr, "wqk": wqk, "wv": wv, "wp": wp, "ident": ident})
    return maps


def kernel(x, w_qkv, w_proj, lambda_q1, lambda_k1, lambda_q2, lambda_k2, rms_scale):
    from concourse.bass_utils import run_bass_kernel_spmd

    x = np.asarray(x, dtype=np.float32)
    w_qkv = np.asarray(w_qkv, dtype=np.float32)
    w_proj = np.asarray(w_proj, dtype=np.float32)
    rms_scale = np.asarray(rms_scale, dtype=np.float32)
    lam1 = np.exp(np.sum(np.asarray(lambda_q1) * np.asarray(lambda_k1), dtype=np.float32))
    lam2 = np.exp(np.sum(np.asarray(lambda_q2) * np.asarray(lambda_k2), dtype=np.float32))
    lam = float(lam1 - lam2 + LAMBDA_INIT)

    nc = build(lam)
    in_maps = _core_inputs(x, w_qkv, w_proj, rms_scale)
    res = run_bass_kernel_spmd(nc, in_maps, core_ids=list(range(N_CORES)))
    y = np.zeros((TT, P, T), np.float32)
    for rmap in res.results:
        y += np.asarray(rmap["y"], np.float32)
    return y.reshape(1, T, C)


# revision 27
# speedup vs baseline: 1.1539x; 1.0376x over previous
"""Trainium2 Bass kernel for DiffSelfAttention (B=1, T=2048, C=2048, 16 v-heads).

Sharding: tensor-parallel over heads across 8 NeuronCores. Core c owns
v-heads {2c, 2c+1} plus the matching q/k heads of both differential branches.
Each core computes its qkv slice, the attention for its 4 q/k head-pairs, the
differential + per-head RMSNorm, and a partial projection
y_c = out_c @ w_proj[rows_c]. The host sums the 8 partials (unshard step).

v2 design notes (vs the fp32r v1):
  - Everything bf16 on the PE (1 cycle/row at ANY moving size, halved DMA
    and SBUF footprint). Host converts inputs; rel-err budget is 2e-2 and
    bf16 keeps us ~1e-2 or better.
  - Transposed PV: attention is computed as a^T[tq,d2] = et^T @ [v|1] with
    the exp'd scores as the STATIONARY operand and [v | ones-column] as a
    129-wide moving operand. This gets the softmax denominator r in the
    same matmul (column 128) AND puts r on the partition axis, so all the
    differential-combine scalars are per-partition [P,1] operands — no
    broadcast matmuls. v1's separate ones-colsum (131k cycles/core) is gone.
  - Softmax divisions eliminated as in v1: RMSNorm is invariant to any
    per-column positive scale, so o' = a1*r2 - lam*a2*r1 feeds the norm.
  - RMS rsqrt = exp(-0.5*ln(mean)) on ACT, batched [P,8] per (block,head)
    to bound Ln/Exp table swaps (Ln and Exp are in different ACT table
    sets on this compiler: per-element ln/exp costs a 1.3us table load).
    rms_scale * (1-lambda_init) is folded into w_proj rows on the host.
  - o'[tq,d2] is transposed back to [d2,tq] for the projection on the PE
    (identity-matmul transpose). DmaTransposeAnt and tensor_tensor_reduce
    both kill the exec unit on this runtime - do not use them.
  - Work is emitted in 2 tq-blocks of 1024; the q projections for the
    second half, the deferred transposes, and the first block's output
    projection are injected as fillers into the (ACT-bound) attention
    sweeps so the PE never idles. Transposes are injected in PAIRS to
    keep the 2-slot scores-psum rotation aligned, and only from k-iter 8
    so their DVE deps are ready.
  - PSUM: one accumulation group per 2KB bank (start=True lazily zeroes
    the whole bank): scores [P,1024]x2 = 4 banks, pv accumulators
    (8 x [P,132] packed 3-per-bank, one group per bank) = 3, proj = 1.
    The tail projection steals the 3 freed accumulator banks.
  - Input DMAs split across the two HWDGE queues (SP: weights,
    ACT: x chunks) so the transfers overlap; ~12us to first matmul.
"""

import math

import numpy as np

import concourse.bass as bass
import concourse.bacc as bacc
import concourse.mybir as mybir
import concourse.tile as tile

F32 = mybir.dt.float32
BF16 = mybir.dt.bfloat16

T = 2048
C = 2048
N_HEAD = 16
H_DIM = 64
D2 = 2 * H_DIM  # 128 (v-head dim, also the RMS group size)
LAMBDA_INIT = 0.8 - 0.6 * math.exp(-0.3)
SCALE = 1.0 / math.sqrt(H_DIM)
P = 128
KS = C // P  # 16 contraction slabs
TT = T // P  # 16 t-tiles
NCH = 512  # phase-1 t-chunk width
QS = 8  # tq slabs per block
NBLK = 2  # tq blocks of 1024
N_CORES = 8

EXP = mybir.ActivationFunctionType.Exp
LOG = mybir.ActivationFunctionType.Ln
CPY = mybir.ActivationFunctionType.Copy
MULT = mybir.AluOpType.mult
ADD = mybir.AluOpType.add


def build(lam: float) -> bass.Bass:
    nc = bacc.Bacc("TRN2", target_bir_lowering=False, debug=False)

    xb_d = nc.dram_tensor("xt", [P, 4, KS, NCH], BF16, kind="ExternalInput")
    wqk_d = nc.dram_tensor("wqk", [P, 4, KS, P], BF16, kind="ExternalInput")
    wv_d = nc.dram_tensor("wv", [P, KS, 2 * D2], BF16, kind="ExternalInput")
    wp_d = nc.dram_tensor("wp", [P, 2, T], BF16, kind="ExternalInput")
    id_d = nc.dram_tensor("ident", [P, P], BF16, kind="ExternalInput")
    y_d = nc.dram_tensor("y", [TT, P, T], BF16, kind="ExternalOutput")

    with tile.TileContext(nc) as tc:
        with tc.tile_pool(name="persist", bufs=1) as pp, \
             tc.tile_pool(name="etp", bufs=4) as etp, \
             tc.tile_pool(name="work", bufs=2) as wkp, \
             tc.tile_pool(name="ysp", bufs=2) as ysp, \
             tc.tile_pool(name="sc", bufs=2, space="PSUM") as scp, \
             tc.tile_pool(name="acc", bufs=1, space="PSUM") as accp, \
             tc.tile_pool(name="yp", bufs=1, space="PSUM") as ypp:

            xb = pp.tile([P, 4, KS, NCH], BF16)
            wqk = pp.tile([P, 4, KS, P], BF16)
            wv = pp.tile([P, KS, 2 * D2], BF16)
            wp = pp.tile([P, 2, T], BF16)
            qk = pp.tile([P, 4, T], BF16)  # m: q1|q2|k1|k2, [d, T] layout
            ident = pp.tile([P, P], BF16)
            vb = pp.tile([P, KS, 2, 130], BF16)  # [tk, kslab, vh, v|1|pad]

            # chunks in consumption order on the ACT queue; weights (then
            # later chunks) on the SP queue — the queues share HBM bandwidth,
            # so loading a late chunk early starves an earlier one
            nc.scalar.dma_start(out=xb[:, 0, 0:8], in_=xb_d[:, 0, 0:8])
            nc.scalar.dma_start(out=xb[:, 0, 8:16], in_=xb_d[:, 0, 8:16])
            nc.scalar.dma_start(out=xb[:, 1], in_=xb_d[:, 1])
            nc.sync.dma_start(out=wqk[:, 2], in_=wqk_d[:, 2])  # k1 first
            nc.sync.dma_start(out=wqk[:, 3], in_=wqk_d[:, 3])
            nc.sync.dma_start(out=wv, in_=wv_d[:])
            nc.sync.dma_start(out=wqk[:, 0], in_=wqk_d[:, 0])
            nc.sync.dma_start(out=wqk[:, 1], in_=wqk_d[:, 1])
            nc.sync.dma_start(out=xb[:, 2], in_=xb_d[:, 2])
            nc.sync.dma_start(out=xb[:, 3], in_=xb_d[:, 3])
            nc.sync.dma_start(out=wp, in_=wp_d[:])
            nc.sync.dma_start(out=ident, in_=id_d[:])
            nc.gpsimd.memset(vb[:, :, :, D2:D2 + 1], 1.0)

            # ---------------- phase 1: qkv projections ----------------
            # (PSUM->SBUF copies ride the ACT engine, idle until the first
            # exp; GPSIMD/Pool cannot read PSUM)
            def emit_qkv_m(n, m, dve_copy=False):
                ps = scp.tile([P, NCH], F32, tag="s", name=f"psq{n}{m}")
                for k in range(KS):
                    nc.tensor.matmul(
                        ps,
                        wqk[:, m, k, :],
                        xb[:, n, k, :],
                        start=(k == 0),
                        stop=(k == KS - 1),
                    )
                if dve_copy:
                    nc.vector.tensor_copy(qk[:, m, n * NCH:(n + 1) * NCH], ps)
                else:
                    nc.scalar.activation(qk[:, m, n * NCH:(n + 1) * NCH], ps,
                                         CPY)

            def emit_v(n, t2):
                g = 4 * n + t2
                ps = scp.tile([P, 2, D2], F32, tag="s", name=f"psv{g}")
                for k in range(KS):
                    nc.tensor.matmul(
                        ps,
                        xb[:, n, k, t2 * P:(t2 + 1) * P],
                        wv[:, k, :],
                        start=(k == 0),
                        stop=(k == KS - 1),
                    )
                nc.scalar.activation(vb[:, g, :, 0:D2], ps, CPY)

            for n in range(4):
                for m in (2, 3):  # k1, k2 (stationaries for all sweeps)
                    emit_qkv_m(n, m)
                for t2 in range(4):
                    emit_v(n, t2)
            for n in (0, 1):
                for m in (0, 1):  # q1, q2 for block 0 (copies on DVE: these
                    # land right when the first sweep's exp stream starts)
                    emit_qkv_m(n, m, dve_copy=True)

            # ---- filler machinery: PE work injected into ACT-bound sweeps
            q_fillers = []  # q projections for tq-block 1 (drain in block 0)
            tp_fillers = []  # deferred PE transposes (gated: DVE deps lag)
            proj_fillers = []  # block-0 output projection (drain in block 1)

            def inject(budget_ns, tp_ok=True):
                while budget_ns > 0:
                    if q_fillers:
                        lst = q_fillers
                    elif tp_fillers:
                        if not tp_ok:
                            return  # keep ordering: proj waits for tps
                        lst = tp_fillers
                    elif proj_fillers:
                        lst = proj_fillers
                    else:
                        return
                    est, f = lst.pop(0)
                    f()
                    budget_ns -= est

            def queue_q_fillers(n, m):
                # q projections for tq-block 1, chunk n, using the yp psum
                # slot (idle during block-0 sweeps)
                box = {}

                def mk(k):
                    def f():
                        if k == 0:
                            box["ps"] = ypp.tile(
                                [P, NCH], F32, tag="y", name=f"psq{n}{m}"
                            )
                        nc.tensor.matmul(
                            box["ps"],
                            wqk[:, m, k, :],
                            xb[:, n, k, :],
                            start=(k == 0),
                            stop=(k == KS - 1),
                        )
                        if k == KS - 1:
                            nc.vector.tensor_copy(
                                qk[:, m, n * NCH:(n + 1) * NCH], box["ps"]
                            )

                    return (220.0, f)

                q_fillers.extend(mk(k) for k in range(KS))

            for n in (2, 3):
                for m in (0, 1):
                    queue_q_fillers(n, m)

            def emit_tp(o2, vh, q, otT, psum_src=None, act_copy=False):
                pool, tag = psum_src if psum_src else (scp, "s")
                pt = pool.tile([P, P], BF16, tag=tag, name=f"tp{vh}{q}")
                nc.tensor.transpose(pt, o2[:, q, :], ident)
                if act_copy:
                    nc.scalar.activation(otT[:, vh, q, :], pt, CPY)
                else:
                    nc.vector.tensor_copy(otT[:, vh, q, :], pt)

            def queue_tp_pairs(o2, vh, otT):
                for q0 in range(0, QS, 2):
                    def f(q0=q0):
                        emit_tp(o2, vh, q0, otT)
                        emit_tp(o2, vh, q0 + 1, otT)
                    tp_fillers.append((120.0, f))

            # ---------------- phase 2: attention sweeps ----------------
            def get_accs(bk, vh, br):
                a = accp.tile([P, 3, 132], F32, tag="accA", name=f"accA{bk}{vh}{br}")
                b = accp.tile([P, 3, 132], F32, tag="accB", name=f"accB{bk}{vh}{br}")
                c = accp.tile([P, 2, 132], F32, tag="accC", name=f"accC{bk}{vh}{br}")
                return [(a, 0), (a, 1), (a, 2), (b, 0), (b, 1), (b, 2), (c, 0), (c, 1)]

            def sweep(bk, vh, br):
                # scores + exp + pv for head-pair (vh,br), tq block bk,
                # software-pipelined one k-slab ahead so the PE never waits
                # a full exp latency
                rows = slice(vh * H_DIM, (vh + 1) * H_DIM)
                accs = get_accs(bk, vh, br)
                ets = [None] * TT
                for j in range(TT + 1):
                    if j < TT:
                        ps = scp.tile(
                            [P, 2 * NCH], F32, tag="s", name=f"sc{bk}{vh}{br}{j}"
                        )
                        for i in range(2):
                            nc.tensor.matmul(
                                ps[:, i * NCH:(i + 1) * NCH],
                                qk[rows, 2 + br, j * P:(j + 1) * P],
                                qk[rows, br, bk * 1024 + i * NCH:
                                   bk * 1024 + (i + 1) * NCH],
                                start=True,
                                stop=True,
                            )
                        et = etp.tile(
                            [P, 2 * NCH], BF16, tag="e", name=f"et{bk}{vh}{br}{j}"
                        )
                        nc.scalar.activation(et, ps, EXP, scale=SCALE)
                        ets[j] = et
                    if j > 0:
                        inject(900.0 if j == 1 else 220.0, tp_ok=(j >= 8))
                        et = ets[j - 1]
                        for q in range(QS):
                            at, qi = accs[q]
                            # one psum accumulation group per BANK: start
                            # zeroes the whole 2KB zero-region lazily
                            first = (j - 1 == 0) and qi == 0
                            lastq = qi == (3 if q < 6 else 2) - 1
                            nc.tensor.matmul(
                                at[:, qi, 0:129],
                                et[:, q * P:(q + 1) * P],
                                vb[:, j - 1, vh, 0:129],
                                start=first,
                                stop=(j - 1 == TT - 1) and lastq,
                            )
                        ets[j - 1] = None
                return accs

            def save_accs(bk, vh, br, accs, tag):
                # copy a|r psum accumulators to SBUF (frees the acc banks
                # for the next sweep after ~3 DVE ops)
                sb = wkp.tile([P, QS, 132], F32, tag=tag, name=f"{tag}{bk}{vh}")
                nc.vector.tensor_copy(sb[:, 0:3, 0:129], accs[0][0][:, :, 0:129])
                nc.scalar.activation(sb[:, 3:6, 0:129], accs[3][0][:, :, 0:129],
                                     CPY)
                nc.vector.tensor_copy(sb[:, 6:8, 0:129], accs[6][0][:, :, 0:129])
                return sb

            SQR = mybir.ActivationFunctionType.Square

            def combine(bk, vh, asb, bsb, otT, tail=False):
                # o' = a1*r2 - lam*a2*r1 (per-column rescale of the true o;
                # RMSNorm cancels it), then per-head RMS + bf16 + transpose.
                r1n = wkp.tile([P, QS, 1], F32, tag="r1n", name=f"r1n{bk}{vh}")
                o12 = wkp.tile([P, QS, P], F32, tag="o12", name=f"o12{bk}{vh}")
                sqs = wkp.tile([P, P], F32, tag="sqs", name=f"sqs{bk}{vh}")
                msb = wkp.tile([P, QS, 1], F32, tag="msb", name=f"msb{bk}{vh}")
                lns = wkp.tile([P, QS, 1], F32, tag="lns", name=f"lns{bk}{vh}")
                rs = wkp.tile([P, QS, 1], F32, tag="rs", name=f"rs{bk}{vh}")
                o2 = wkp.tile([P, QS, P], BF16, tag="o2", name=f"o2{bk}{vh}")
                nc.vector.tensor_scalar_mul(r1n, asb[:, :, 128:129], -lam)

                I32 = mybir.dt.int32
                SHR = mybir.AluOpType.logical_shift_right

                def rms_tail(qs):
                    # rs = rsqrt(msb) via bit-hack seed + 2 Newton steps
                    nc.vector.tensor_scalar(
                        out=lns[:, qs, :].bitcast(I32),
                        in0=msb[:, qs, :].bitcast(I32),
                        scalar1=1, scalar2=None, op0=SHR)
                    nc.vector.tensor_scalar(
                        out=rs[:, qs, :].bitcast(I32),
                        in0=lns[:, qs, :].bitcast(I32),
                        scalar1=-1, scalar2=0x5F3759DF, op0=MULT, op1=ADD)
                    for _ in range(2):
                        nc.vector.tensor_mul(lns[:, qs, :], rs[:, qs, :],
                                             rs[:, qs, :])
                        nc.vector.tensor_mul(lns[:, qs, :], lns[:, qs, :],
                                             msb[:, qs, :])
                        nc.vector.tensor_scalar(
                            out=lns[:, qs, :], in0=lns[:, qs, :],
                            scalar1=-0.5, scalar2=1.5, op0=MULT, op1=ADD)
                        nc.vector.tensor_mul(rs[:, qs, :], rs[:, qs, :],
                                             lns[:, qs, :])

                for q in range(QS):
                    nc.vector.tensor_scalar_mul(
                        o12[:, q, :], asb[:, q, 0:P], bsb[:, q, 128:129]
                    )
                    nc.vector.scalar_tensor_tensor(
                        o12[:, q, :], bsb[:, q, 0:P], r1n[:, q, :], o12[:, q, :],
                        op0=MULT, op1=ADD,
                    )
                    if tail:
                        nc.scalar.activation(sqs, o12[:, q, :], SQR,
                                             accum_out=msb[:, q, :])
                    else:
                        nc.vector.tensor_mul(sqs, o12[:, q, :], o12[:, q, :])
                        nc.vector.tensor_reduce(
                            msb[:, q, :], sqs, mybir.AxisListType.X, ADD
                        )
                rms_tail(slice(0, QS))
                for q in range(QS):
                    if tail and q % 2 == 0:
                        nc.scalar.activation(o2[:, q, :], o12[:, q, :], CPY,
                                             scale=rs[:, q, :])
                    else:
                        nc.vector.tensor_scalar_mul(
                            o2[:, q, :], o12[:, q, :], rs[:, q, :]
                        )
                queue_tp_pairs(o2, vh, otT)

            # -------- output projection for one 128-row tq tile ----------
            # psum_src: (pool, tag) for this tile's psum bank. Mid-kernel
            # (filler path) only the 1-bank "y" tag is free and copies go to
            # DVE; at the tail the attention accumulator banks are stolen
            # for a 4-bank rotation and copies split DVE/ACT.
            def proj_tile_closures(otT, t, psum_src, split_copies):
                q = t % QS
                pool, tag = psum_src
                box = {}
                cl = []

                def c_vh0(p):
                    def f():
                        if p == 0:
                            box["yp"] = pool.tile([P, 2, 256], F32, tag=tag,
                                                  name=f"yt{t}")
                            box["ys"] = ysp.tile([P, 4, 2, 256], BF16,
                                                 tag="ysb", name=f"ys{t}")
                        yp = box["yp"]
                        for r in range(2):
                            nc.tensor.matmul(
                                yp[:, r, :],
                                otT[:, 0, q, :],
                                wp[:, 0, 512 * p + 256 * r:512 * p + 256 * (r + 1)],
                                start=(r == 0),
                                stop=False,
                            )
                    return (230.0, f)

                def c_vh1(p):
                    def f():
                        yp = box["yp"]
                        for r in range(2):
                            nc.tensor.matmul(
                                yp[:, r, :],
                                otT[:, 1, q, :],
                                wp[:, 1, 512 * p + 256 * r:512 * p + 256 * (r + 1)],
                                start=False,
                                stop=(r == 1),
                            )
                        if split_copies:
                            nc.vector.tensor_copy(box["ys"][:, p, 0, :],
                                                  yp[:, 0, :])
                            nc.scalar.activation(box["ys"][:, p, 1, :],
                                                 yp[:, 1, :], CPY)
                        else:
                            nc.vector.tensor_copy(box["ys"][:, p, :, :], yp)
                    return (230.0, f)

                def c_dma():
                    nc.sync.dma_start(out=y_d[t], in_=box["ys"])

                for p in range(4):
                    cl.append(c_vh0(p))
                    cl.append(c_vh1(p))
                cl.append((0.0, c_dma))
                return cl

            # ---------------- blocks ----------------
            tail_rot = [(ypp, "y"), (accp, "accA"), (accp, "accB"),
                        (accp, "accC")]
            for bk in range(NBLK):
                if bk == 1:
                    # block-1 scores read the filler-produced q projections:
                    # force-drain any q fillers that block 0 didn't absorb
                    while q_fillers:
                        q_fillers.pop(0)[1]()
                otT = wkp.tile([P, 2, QS, P], BF16, tag="otT", name=f"otT{bk}")
                last = bk == NBLK - 1
                for vh in range(2):
                    accs0 = sweep(bk, vh, 0)
                    asb = save_accs(bk, vh, 0, accs0, "asb")
                    accs1 = sweep(bk, vh, 1)
                    bsb = save_accs(bk, vh, 1, accs1, "bsb")
                    combine(bk, vh, asb, bsb, otT, tail=(last and vh == 1))
                if not last:
                    for t in range(QS):
                        proj_fillers.extend(
                            proj_tile_closures(otT, bk * QS + t, (ypp, "y"),
                                               split_copies=False))
                else:
                    inject(1e9)  # drain leftovers (incl. final transposes)
                    for t in range(QS):
                        for est, f in proj_tile_closures(
                                otT, bk * QS + t, tail_rot[t % 4],
                                split_copies=True):
                            f()
    nc.finalize()
    return nc


def _core_inputs(x, w_qkv, w_proj, rms_scale):
    """Host-side shard prep: per-core bf16 weight slices + replicated x^T."""
    bf = mybir.dt.np(BF16)
    ident = np.ascontiguousarray(np.eye(P, dtype=np.float32).astype(bf))
    xt = x.reshape(T, C).T  # [C, T]
    xtr = np.ascontiguousarray(
        xt.reshape(KS, P, 4, NCH).transpose(1, 2, 0, 3).astype(bf)
    )
    sv = np.tile(
        rms_scale.astype(np.float32) * np.float32(1.0 - LAMBDA_INIT)
        * np.float32(math.sqrt(D2)), 2
    )  # [256]; sqrt(D2) because the kernel's rsqrt takes the SUM of squares
    maps = []
    for c in range(N_CORES):
        cols = [
            w_qkv[:, 0 * 1024 + c * P:0 * 1024 + (c + 1) * P],  # q1 heads 2c,2c+1
            w_qkv[:, 1 * 1024 + c * P:1 * 1024 + (c + 1) * P],  # q2
            w_qkv[:, 2 * 1024 + c * P:2 * 1024 + (c + 1) * P],  # k1
            w_qkv[:, 3 * 1024 + c * P:3 * 1024 + (c + 1) * P],  # k2
        ]
        wqk = np.stack(cols, axis=0)  # [4, C, 128]
        wqk = np.ascontiguousarray(
            wqk.reshape(4, KS, P, P).transpose(2, 0, 1, 3).astype(bf)
        )
        wv = w_qkv[:, 2 * C + c * 2 * D2:2 * C + (c + 1) * 2 * D2]  # [C, 256]
        wv = np.ascontiguousarray(
            wv.reshape(KS, P, 2 * D2).transpose(1, 0, 2).astype(bf)
        )
        wp = w_proj[c * 2 * D2:(c + 1) * 2 * D2, :] * sv[:, None]  # [256, T]
        wp = np.ascontiguousarray(
            wp.reshape(2, P, T).transpose(1, 0, 2).astype(bf)
        )
        maps.append({"xt": xtr, "wqk": wqk, "wv": wv, "wp": wp, "ident": ident})
    return maps


def kernel(x, w_qkv, w_proj, lambda_q1, lambda_k1, lambda_q2, lambda_k2, rms_scale):
    from concourse.bass_utils import run_bass_kernel_spmd

    x = np.asarray(x, dtype=np.float32)
    w_qkv = np.asarray(w_qkv, dtype=np.float32)
    w_proj = np.asarray(w_proj, dtype=np.float32)
    rms_scale = np.asarray(rms_scale, dtype=np.float32)
    lam1 = np.exp(np.sum(np.asarray(lambda_q1) * np.asarray(lambda_k1), dtype=np.float32))
    lam2 = np.exp(np.sum(np.asarray(lambda_q2) * np.asarray(lambda_k2), dtype=np.float32))
    lam = float(lam1 - lam2 + LAMBDA_INIT)

    nc = build(lam)
    in_maps = _core_inputs(x, w_qkv, w_proj, rms_scale)
    res = run_bass_kernel_spmd(nc, in_maps, core_ids=list(range(N_CORES)))
    y = np.zeros((TT, P, T), np.float32)
    for rmap in res.results:
        y += np.asarray(rmap["y"], np.float32)
    return y.reshape(1, T, C)
